# revision 18
# baseline (speedup 1.0000x reference)
"""MetaOptNet SVM-CS head on 8 Trainium2 NeuronCores.

Math: the reference runs a 15-iteration Mehrotra interior-point solve of the
Crammer-Singer dual QP per task. Empirically (f64 replication) the IPM is
fully converged by iteration 15, so the target equals the QP optimum. We
compute that optimum with a fixed-matrix ADMM:

    per task:  K = S S^T  (25x25 Gram)
               W~ = rho * (K + (1+rho) I)^{-1}   (Newton-Schulz, 3 bf16 iters)
               10x ADMM (rho=8), in (d1 = u-y, oy = y+oh/rho) state form:
                   t = center_ways(W~ @ d1) + oy
                   d1' = min(t, 2h - t);  oy' = max(t - (h - oh/rho), oh/rho)
                   where h = (C + 1/rho) oh
               logits = Q @ (S^T x) * scale    (x = center_ways(W~ @ d1))

The equality constraint A z = 0 reduces to centering across ways because
A A^T = n_way I; the KKT matrix is way-block-diagonal with identical blocks
K + (1+rho)I, which is what makes one 25x25 inverse per task sufficient.

Sharding: pure data parallel, 16 tasks per core. Host-side work is layout
only (shard, transpose packing into 128-partition DMA tiles, one-hot
constants); all FLOPs run on-device.

PE-efficiency notes (the real HW bottleneck is LDWEIGHTS time, which the
CoreSim cost model does not model):
 - Gram: S^T shipped fp8-e3m4 (x64 prescale, descale folded into the mask
   const) in 32-col-strided task windows [128, 128] so one FWL weight load
   covers a 4-task group; the full-window matmul leaves junk in cross-task
   blocks which the mask zeroes before Newton-Schulz.
 - Stage 4 (w = S^T x): x is expanded to a block-diagonal [128, 20] tile
   (xdiag) so a single 128-col FWL load of S serves all 4 tasks of a group.
 - Stage 5 (logits = Q w): Q rides as stationary 128-col FWL windows over
   the task-concatenated query axis; the w vectors (5 cols/task) move.
   Logits come out in window-row layout [128, 125]; the host re-assembles.

Precision: QP in fp32 (ADMM) with bf16 Newton-Schulz; S/Q contracted in
bf16 except the Gram (fp8-e3m4). Measured end-to-end ~5e-3 relative
(tolerance 2e-2).
"""

import sys

sys.path.insert(0, "/opt/trn_rl_repo")

from contextlib import ExitStack

import numpy as np
import ml_dtypes

import concourse.bass as bass
import concourse.tile as tile
from concourse import mybir
from concourse.alu_op_type import AluOpType
from concourse.bass_utils import run_bass_kernel_spmd
from concourse.tile import TileContext

# ---------------------------------------------------------------------------
# Problem constants (hardcoded per the harness contract)
N_CORES = 8
B_TOT = 128
T = 16            # tasks per core
NS = 25           # support samples per task
NW = 5            # ways
NQ = 75           # queries per task
D = 2560          # feature dim
NCH = D // 128    # 20 d-chunks
G = 4             # task groups per core (4 tasks each -> 128-col windows)
GP = T // G       # tasks per group
RHO = 8.0
NS_C = 0.065      # Newton-Schulz init scale for H = K + 9I
NS_ITERS = 3
ADMM_ITERS = 10
C_REG = 0.1
GRAM_E3 = True    # ship S^T (Gram operand) as fp8-e3m4 (else bf16)
E3SCALE = 64.0 if GRAM_E3 else 1.0  # prescale; descaled via mask const
Q_E3 = True       # ship Q as fp8-e3m4 (x64, folded into the output scale)
QSCALE = 64.0 if Q_E3 else 1.0
NQT = 1280        # query cols padded to 10 x 128 windows
WIN = NQT // 128

F32 = mybir.dt.float32
BF16 = mybir.dt.bfloat16
FP8E3 = mybir.dt.float8e3 if GRAM_E3 else mybir.dt.bfloat16
QDT = mybir.dt.float8e3 if Q_E3 else mybir.dt.bfloat16


def _win_map():
    """Stage-5 window map: [(q0, q1, t0, t1, segs, colbase)] and total cols.
    segs: list of (g, ta, tb) group-contiguous task runs."""
    wins = []
    base = 0
    for w in range(WIN):
        q0 = w * 128
        q1 = min(q0 + 128, T * NQ)
        if q0 >= T * NQ:
            break
        t0, t1 = q0 // NQ, (q1 - 1) // NQ
        segs = []
        ta = t0
        while ta <= t1:
            g = ta // GP
            tb = min(t1, (g + 1) * GP - 1)
            segs.append((g, ta, tb))
            ta = tb + 1
        wins.append((q0, q1, t0, t1, segs, base))
        base += (t1 - t0 + 1) * NW
    return wins, base


WINDOWS, OUT_COLS = _win_map()


# ---------------------------------------------------------------------------
# The walrus build here encodes at most ONE sync-wait command per instruction
# (TPB_CTRL / S3_LW setupSyncWait raises "Too many sync wait commands").
# Tile's scheduler freely attaches several waits to one instruction, so after
# scheduling we split the excess onto NoOps inserted immediately before the
# instruction on the same engine — identical semantics, encodable waits.
def _split_waits(nc, max_waits=1):
    cnt = 0
    for blk in nc.m.functions[0].blocks:
        insns = blk.instructions
        idx = 0
        while idx < len(insns):
            ins = insns[idx]
            si = ins.sync_info
            waits = list(si.on_wait) if si and si.on_wait else []
            if len(waits) > max_waits:
                si.on_wait = waits[:max_waits]
                for w in waits[max_waits:]:
                    nop = mybir.InstNoOp(name=f"waitnop_{cnt}", ins=[], outs=[])
                    cnt += 1
                    nop.engine = ins.engine
                    nop.sync_info = mybir.SyncInfo(on_wait=[w], on_update=[])
                    nc.register_instruction(nop, overwrite=True)
                    insns.insert(idx, nop)
                    idx += 1
            idx += 1
    return cnt


# ---------------------------------------------------------------------------
def _build_program(repeat: int = 1, unroll: int = 1, variant: str = "full"):
    """repeat>1 wraps the whole body in a hardware loop executing it that many
    times per launch — used by test.py to measure per-iteration device time as
    a slope, cancelling the (fixed, ~70ms) axon dispatch round-trip. The
    graded kernel() path always uses repeat=1, unroll=1, variant="full".

    unroll emits the body that many times inside the loop (separates loop
    overhead from body time). variant: "full" | "dma_only" (loads + store
    only) | "compute_only" (loads hoisted out of the loop)."""
    nc = bass.Bass("TRN2", target_bir_lowering=False)

    st_d = nc.dram_tensor("st", [NCH, 128, G * 128], FP8E3, kind="ExternalInput")
    sn_d = nc.dram_tensor("sn", [G, 128, D], BF16, kind="ExternalInput")
    qt_d = nc.dram_tensor("qt", [NCH, 128, NQT], QDT, kind="ExternalInput")
    ohc_d = nc.dram_tensor("ohc", [128, 20], F32, kind="ExternalInput")
    h2_d = nc.dram_tensor("h2", [128, 20], F32, kind="ExternalInput")
    hmo_d = nc.dram_tensor("hmo", [128, 20], F32, kind="ExternalInput")
    i2_d = nc.dram_tensor("i2", [128, 128], F32, kind="ExternalInput")
    cib_d = nc.dram_tensor("cib", [128, 128], BF16, kind="ExternalInput")
    nine_d = nc.dram_tensor("nine", [128, 128], F32, kind="ExternalInput")
    maskq_d = nc.dram_tensor("maskq", [128, 128], F32, kind="ExternalInput")
    scale_d = nc.dram_tensor("scale", [1, 1], F32, kind="ExternalInput")
    out_d = nc.dram_tensor("out", [128, OUT_COLS], F32, kind="ExternalOutput")
    if variant == "debug":
        dbg_h = nc.dram_tensor("dbg_h", [128, 128], F32, kind="ExternalOutput")
        dbg_wt = nc.dram_tensor("dbg_wt", [128, 128], F32, kind="ExternalOutput")
        dbg_xb = nc.dram_tensor("dbg_xb", [128, 20], BF16, kind="ExternalOutput")
        dbg_w = nc.dram_tensor("dbg_w", [128, NCH * GP * NW], BF16, kind="ExternalOutput")

    with ExitStack() as ctx:
        tc = ctx.enter_context(TileContext(nc))
        st_pool = ctx.enter_context(tc.tile_pool(name="st", bufs=1))
        sn_pool = ctx.enter_context(tc.tile_pool(name="sn", bufs=G))
        qt_pool = ctx.enter_context(tc.tile_pool(name="qt", bufs=NCH))
        consts = ctx.enter_context(tc.tile_pool(name="consts", bufs=1))
        mats = ctx.enter_context(tc.tile_pool(name="mats", bufs=12))
        state = ctx.enter_context(tc.tile_pool(name="state", bufs=14))
        wout = ctx.enter_context(tc.tile_pool(name="wout", bufs=4))

        def emit_loads(all_qt=False):
            # NS-critical consts first on the Act queue
            i2_sb = consts.tile([128, 128], F32, tag="i2")
            nc.scalar.dma_start(out=i2_sb, in_=i2_d[:, :])
            cib_sb = consts.tile([128, 128], BF16, tag="cib")
            nc.scalar.dma_start(out=cib_sb, in_=cib_d[:, :])
            nine_sb = consts.tile([128, 128], F32, tag="nine")
            nc.scalar.dma_start(out=nine_sb, in_=nine_d[:, :])
            maskq_sb = consts.tile([128, 128], F32, tag="maskq")
            nc.scalar.dma_start(out=maskq_sb, in_=maskq_d[:, :])
            st_tile = st_pool.tile([128, NCH * G * 128], FP8E3, tag="st")
            for j in range(4):
                nch4 = NCH // 4
                eng = nc.sync if j % 2 == 0 else nc.scalar
                eng.dma_start(
                    out=st_tile[:, j * nch4 * G * 128 : (j + 1) * nch4 * G * 128],
                    in_=st_d[j * nch4 : (j + 1) * nch4, :, :],
                )
            st_sb = [
                st_tile[:, c * G * 128 : (c + 1) * G * 128] for c in range(NCH)
            ]
            sn_sb = []
            for g in range(G):
                t_ = sn_pool.tile([128, D], BF16, tag="sn")
                nc.sync.dma_start(out=t_, in_=sn_d[g, :, :])
                sn_sb.append(t_)
            ohc_sb = consts.tile([128, 20], F32, tag="ohc")
            nc.scalar.dma_start(out=ohc_sb, in_=ohc_d[:, :])
            h2_sb = consts.tile([128, 20], F32, tag="h2")
            nc.scalar.dma_start(out=h2_sb, in_=h2_d[:, :])
            hmo_sb = consts.tile([128, 20], F32, tag="hmo")
            nc.scalar.dma_start(out=hmo_sb, in_=hmo_d[:, :])
            scale_sb = consts.tile([128, 1], F32, tag="scale")
            nc.scalar.dma_start(out=scale_sb, in_=scale_d[:, :].to_broadcast([128, 1]))

            # ADMM state: d1 = u - y (init ohc), oy = y + ohc (init ohc)
            d1_sb = state.tile([128, 20], F32, tag="d1")
            nc.scalar.dma_start(out=d1_sb, in_=ohc_d[:, :])
            oy_sb = state.tile([128, 20], F32, tag="oy")
            nc.scalar.dma_start(out=oy_sb, in_=ohc_d[:, :])

            qt_sb = []
            n_pre = NCH if all_qt else NCH // 2
            for c in range(NCH):
                t_ = qt_pool.tile([128, NQT], QDT, tag="qt")
                if c < n_pre:
                    nc.scalar.dma_start(out=t_, in_=qt_d[c, :, :])
                qt_sb.append(t_)
            return dict(i2=i2_sb, cib=cib_sb, nine=nine_sb, maskq=maskq_sb,
                        st=st_sb, sn=sn_sb, ohc=ohc_sb, h2=h2_sb, hmo=hmo_sb,
                        scale=scale_sb, d1=d1_sb, oy=oy_sb, qt=qt_sb)

        def emit_compute(hd):
            st_sb, sn_sb, qt_sb = hd["st"], hd["sn"], hd["qt"]
            i2_sb, cib_sb, nine_sb, maskq_sb = (
                hd["i2"], hd["cib"], hd["nine"], hd["maskq"])
            ohc_sb, h2_sb, hmo_sb, scale_sb = (
                hd["ohc"], hd["h2"], hd["hmo"], hd["scale"])
            d1_sb, oy_sb = hd["d1"], hd["oy"]

            # ---- stage 1: K = S S^T per 4-task 128-col window (fp8) -------
            # One FWL weight load per (group, chunk); cross-task junk blocks
            # are zeroed by maskq, which also folds in the 1/E3SCALE^2.
            hb_all = []
            with tc.tile_pool(name="kpsum", bufs=4, space="PSUM") as kpsum:
                kp_all = []
                for g in range(G):
                    kp = kpsum.tile([128, 128], F32, tag="kp")
                    for c in range(NCH):
                        nc.tensor.matmul(
                            kp,
                            lhsT=st_sb[c][:, g * 128 : (g + 1) * 128],
                            rhs=st_sb[c][:, g * 128 : (g + 1) * 128],
                            start=(c == 0),
                            stop=(c == NCH - 1),
                        )
                    kp_all.append(kp)
                h_all = []
                for g in range(G):
                    km = mats.tile([128, 128], F32, tag="km")
                    nc.vector.tensor_tensor(km, kp_all[g], maskq_sb, op=AluOpType.mult)
                    h_sb = mats.tile([128, 128], F32, tag="h")
                    nc.vector.tensor_tensor(h_sb, km, nine_sb, op=AluOpType.add)
                    h_all.append(h_sb)
                    hb = mats.tile([128, 128], BF16, tag="hb")
                    nc.vector.tensor_copy(hb, h_sb)
                    hb_all.append(hb)

            # ---- stage 2: Newton-Schulz inverse, all-bf16 -----------------
            # iters 0..n-2 in bf16 (NS self-corrects), final iter fp32 squares
            # the bf16 error away, so W~ is fp32-quality at reduced PE cost.
            wt_sb = []
            with tc.tile_pool(name="npsum", bufs=4, space="PSUM") as npsum:
                x_cur = [cib_sb] * G
                for it in range(NS_ITERS):
                    last = it == NS_ITERS - 1
                    prev_last = it == NS_ITERS - 2
                    for g in range(G):
                        t1p = npsum.tile([128, 128], F32, tag="t1p")
                        nc.tensor.matmul(
                            t1p,
                            lhsT=h_all[g] if last else hb_all[g],
                            rhs=x_cur[g],
                            start=True,
                            stop=True,
                        )
                        u_ns = mats.tile(
                            [128, 128], F32 if last else BF16,
                            tag="u_ns" if last else "u_nsb",
                        )
                        nc.vector.tensor_tensor(u_ns, i2_sb, t1p, op=AluOpType.subtract)
                        x2p = npsum.tile([128, 128], F32, tag="x2p")
                        nc.tensor.matmul(
                            x2p, lhsT=x_cur[g], rhs=u_ns, start=True, stop=True
                        )
                        if last:
                            wt = mats.tile([128, 128], F32, tag="wt")
                            nc.scalar.activation(
                                wt, x2p, mybir.ActivationFunctionType.Copy, scale=RHO
                            )
                            wt_sb.append(wt)
                        else:
                            x_next = mats.tile(
                                [128, 128], F32 if prev_last else BF16,
                                tag="x_ns" if prev_last else "x_nsb",
                            )
                            nc.scalar.activation(
                                x_next, x2p, mybir.ActivationFunctionType.Copy
                            )
                            x_cur[g] = x_next

            # ---- stage 3: ADMM (d1/oy state form) -------------------------
            # t = center(Wt @ d1) + y + OHC
            # d1' = min(t, 2h-t);  oy' = max(t-(h-OHC), OHC)
            xb_sb = None
            with ExitStack() as pctx:
                mpsum = pctx.enter_context(
                    tc.tile_pool(name="mpsum", bufs=2, space="PSUM"))
                for it in range(ADMM_ITERS):
                    xp = mpsum.tile([128, 20], F32, tag="mp")
                    for g in range(G):
                        nc.tensor.matmul(
                            xp[:, g * NW : (g + 1) * NW],
                            lhsT=wt_sb[g],
                            rhs=d1_sb[:, g * NW : (g + 1) * NW],
                            start=True,
                            stop=True,
                        )
                    msum = state.tile([128, 4], F32, tag="msum")
                    nc.vector.reduce_sum(
                        msum,
                        xp[:, :].rearrange("p (g w) -> p g w", w=NW),
                        axis=mybir.AxisListType.X,
                    )
                    msb = msum[:, :]
                    msb_ap = bass.AP(
                        tensor=msb.tensor, offset=msb.offset,
                        ap=[msb.ap[0], msb.ap[1], [0, NW]],
                    )
                    p1 = state.tile([128, 20], F32, tag="p1")
                    nc.vector.tensor_tensor(p1, xp, oy_sb, op=AluOpType.add)
                    tt_sb = state.tile([128, 20], F32, tag="tt")
                    nc.vector.scalar_tensor_tensor(
                        out=tt_sb[:, :].rearrange("p (g w) -> p g w", w=NW),
                        in0=msb_ap,
                        scalar=-1.0 / NW,
                        in1=p1[:, :].rearrange("p (g w) -> p g w", w=NW),
                        op0=AluOpType.mult,
                        op1=AluOpType.add,
                    )
                    if it == ADMM_ITERS - 1:
                        xb_sb = state.tile([128, 20], BF16, tag="xb")
                        nc.vector.scalar_tensor_tensor(
                            out=xb_sb[:, :].rearrange("p (g w) -> p g w", w=NW),
                            in0=msb_ap,
                            scalar=-1.0 / NW,
                            in1=xp[:, :].rearrange("p (g w) -> p g w", w=NW),
                            op0=AluOpType.mult,
                            op1=AluOpType.add,
                        )
                    n2h = state.tile([128, 20], F32, tag="n2h")
                    nc.vector.scalar_tensor_tensor(
                        out=n2h,
                        in0=tt_sb,
                        scalar=-1.0,
                        in1=h2_sb,
                        op0=AluOpType.mult,
                        op1=AluOpType.add,
                    )
                    d1_sb = state.tile([128, 20], F32, tag="d1n")
                    nc.vector.tensor_tensor(d1_sb, tt_sb, n2h, op=AluOpType.min)
                    if it < ADMM_ITERS - 1:
                        pa = state.tile([128, 20], F32, tag="pa")
                        nc.vector.tensor_tensor(pa, tt_sb, hmo_sb, op=AluOpType.subtract)
                        oy_sb = state.tile([128, 20], F32, tag="oy2")
                        nc.vector.tensor_tensor(oy_sb, pa, ohc_sb, op=AluOpType.max)

                if variant != "compute_only":
                    for c in range(NCH // 2, NCH):
                        nc.scalar.dma_start(out=qt_sb[c], in_=qt_d[c, :, :])

                # ---- stage 4: w = S^T x via block-diagonal xdiag ----------
                # One [128,128] FWL load of S per (group, chunk) serves all
                # 4 tasks: xdiag has task tp's x block at rows tp*32+s,
                # cols tp*5+w, zero elsewhere.
                w_sb_g = []
                with tc.tile_pool(name="wpsum", bufs=4, space="PSUM") as wpsum:
                    for g in range(G):
                        xdiag = state.tile([128, 20], BF16, tag="xdiag")
                        nc.vector.memset(xdiag, 0.0)
                        for tp in range(GP):
                            sl = slice(tp * 32, tp * 32 + NS)
                            nc.vector.tensor_copy(
                                xdiag[sl, tp * NW : (tp + 1) * NW],
                                xb_sb[sl, g * NW : (g + 1) * NW],
                            )
                        wp = wpsum.tile([128, NCH * GP * NW], F32, tag="wp")
                        for c in range(NCH):
                            nc.tensor.matmul(
                                wp[:, c * GP * NW : (c + 1) * GP * NW],
                                lhsT=sn_sb[g][:, c * 128 : (c + 1) * 128],
                                rhs=xdiag,
                                start=True,
                                stop=True,
                            )
                        w_sb = wout.tile([128, NCH * GP * NW], BF16, tag="w")
                        nc.vector.tensor_copy(w_sb, wp)
                        w_sb_g.append(w_sb)

                # ---- stage 5: logits via Q-stationary 128-col windows -----
                # lwin[w][r, 5j+v] = sum_d qt[d, w*128+r] * w_task(t0+j)[d, v]
                out_sb = consts.tile([128, OUT_COLS], F32, tag="outsb")
                with tc.tile_pool(name="lpsum", bufs=1, space="PSUM") as lpsum:
                    lp_all = lpsum.tile([128, OUT_COLS], F32, tag="lw")
                    lwin = []
                    for (q0, q1, t0, t1, segs, base) in WINDOWS:
                        lw_t = lp_all[:, base : base + (t1 - t0 + 1) * NW]
                        lwin.append(lw_t)
                    # window-outer / chunk-inner: PSUM accumulation groups
                    # must not interleave within a bank, so each window's
                    # 20-chunk accumulation completes before the next starts.
                    # matmul start=True clears has_written for the WHOLE psum
                    # bank (data intact), so each window's accumulation must
                    # finish before the next window's start, and only the
                    # first seg of c==0 may carry start=True (the other seg's
                    # region was just cleared, so its first write overwrites).
                    for wi, (q0, q1, t0, t1, segs, base) in enumerate(WINDOWS):
                        for c in range(NCH):
                            for si, (g, ta, tb) in enumerate(segs):
                                nc.tensor.matmul(
                                    lwin[wi][:, (ta - t0) * NW : (tb - t0 + 1) * NW],
                                    lhsT=qt_sb[c][:, q0 : q0 + 128],
                                    rhs=w_sb_g[g][
                                        :,
                                        c * GP * NW + (ta - g * GP) * NW :
                                        c * GP * NW + (tb - g * GP + 1) * NW,
                                    ],
                                    start=(c == 0 and si == 0),
                                    stop=(c == NCH - 1),
                                )
                    for wi, (q0, q1, t0, t1, segs, base) in enumerate(WINDOWS):
                        ncols = (t1 - t0 + 1) * NW
                        nc.scalar.activation(
                            out_sb[:, base : base + ncols],
                            lwin[wi],
                            mybir.ActivationFunctionType.Copy,
                            scale=scale_sb,
                        )
                nc.sync.dma_start(out=out_d[:, :], in_=out_sb)
                if variant == "debug":
                    nc.sync.dma_start(out=dbg_h[:, :], in_=h_all[0])
                    nc.sync.dma_start(out=dbg_wt[:, :], in_=wt_sb[0])
                    nc.sync.dma_start(out=dbg_xb[:, :], in_=xb_sb)
                    nc.sync.dma_start(out=dbg_w[:, :], in_=w_sb_g[0])

        def emit_body():
            if variant == "dma_only":
                emit_loads(all_qt=True)
                zt = consts.tile([128, OUT_COLS], F32, tag="outsb")
                nc.vector.memset(zt, 0.0)
                nc.sync.dma_start(out=out_d[:, :], in_=zt)
            else:
                hd = emit_loads()
                emit_compute(hd)

        if variant == "compute_only":
            hd0 = emit_loads(all_qt=True)
        if repeat > 1:
            ctx.enter_context(tc.For_i(0, repeat, 1))
        for _ in range(unroll):
            if variant == "compute_only":
                emit_compute(hd0)
            else:
                emit_body()

    _split_waits(nc)
    return nc


_NC_CACHE = None


def _get_nc():
    global _NC_CACHE
    if _NC_CACHE is None:
        _NC_CACHE = _build_program()
    return _NC_CACHE


# ---------------------------------------------------------------------------
def _host_prep(support, query, support_labels, scale):
    """Shard + pack into the DMA layouts. Layout only, no FLOPs."""
    f32 = np.float32
    bf = mybir.dt.np(BF16)
    e3 = mybir.dt.np(FP8E3)
    eye = np.eye(NS, dtype=f32)
    blockdiag = np.zeros((128, 128), dtype=f32)
    for tp in range(GP):
        blockdiag[tp * 32 : tp * 32 + NS, tp * 32 : tp * 32 + NS] = eye
    i2 = np.ascontiguousarray(2.0 * blockdiag, dtype=f32)
    ci = np.ascontiguousarray(NS_C * blockdiag, dtype=f32)
    nine = np.ascontiguousarray((1.0 + RHO) * blockdiag, dtype=f32)
    # 25x25 all-ones blocks at 32-spacing, folding in the e3m4 descale
    maskq = np.zeros((128, 128), dtype=f32)
    for tp in range(GP):
        for tq in range(GP):
            if tp == tq:
                maskq[tp * 32 : tp * 32 + NS, tp * 32 : tp * 32 + NS] = (
                    1.0 / (E3SCALE * E3SCALE)
                )
    sc = np.asarray(scale, dtype=f32).reshape(1, 1) / QSCALE

    in_maps = []
    for core in range(N_CORES):
        sl = slice(core * T, (core + 1) * T)
        S = np.asarray(support[sl], dtype=f32)        # [16,25,2560]
        Q = np.asarray(query[sl], dtype=f32)          # [16,75,2560]
        lab = np.asarray(support_labels[sl])          # [16,25] int
        # st: S^T x E3SCALE in 32-col strides: [NCH, 128, g*128 + tp*32 + s]
        s64 = np.clip(S * E3SCALE, -15.5, 15.5)
        stp = np.zeros((NCH, 128, G, GP, 32), dtype=f32)
        stp[:, :, :, :, :NS] = (
            s64.reshape(G, GP, NS, NCH, 128).transpose(3, 4, 0, 1, 2)
        )
        st = np.ascontiguousarray(
            stp.reshape(NCH, 128, G * 128).astype(e3)
        )
        sn = np.zeros((G, 128, D), dtype=bf)
        for tp in range(GP):
            sn[:, tp * 32 : tp * 32 + NS, :] = S.reshape(G, GP, NS, D)[:, tp].astype(bf)
        qtp = np.zeros((NCH, 128, NQT), dtype=f32)
        qtp[:, :, : T * NQ] = Q.transpose(2, 0, 1).reshape(NCH, 128, T * NQ)
        if Q_E3:
            qt = np.ascontiguousarray(
                np.clip(qtp * QSCALE, -15.5, 15.5).astype(mybir.dt.np(QDT))
            )
        else:
            qt = np.ascontiguousarray(qtp.astype(bf))
        oh = (lab[:, :, None] == np.arange(NW)[None, None, :]).astype(f32)
        # [16,25,5] -> [128,20]: row = tp*32+s, col = g*5+w
        ohm = np.zeros((128, 20), dtype=f32)
        ohr = oh.reshape(G, GP, NS, NW).transpose(1, 2, 0, 3).reshape(GP, NS, 20)
        for tp in range(GP):
            ohm[tp * 32 : tp * 32 + NS, :] = ohr[tp]
        in_maps.append(
            {
                "st": st,
                "sn": sn,
                "qt": qt,
                "ohc": np.ascontiguousarray(ohm / RHO),
                "h2": np.ascontiguousarray(2.0 * (C_REG + 1.0 / RHO) * ohm),
                "hmo": np.ascontiguousarray(C_REG * ohm),
                "i2": i2,
                "cib": np.ascontiguousarray(ci.astype(bf)),
                "nine": nine,
                "maskq": maskq,
                "scale": sc,
            }
        )
    return in_maps


def _unshard_out(o):
    """[128, OUT_COLS] window layout -> [T, NQ, NW]."""
    logits = np.zeros((T, NQ, NW), dtype=np.float32)
    for (q0, q1, t0, t1, segs, base) in WINDOWS:
        for t in range(t0, t1 + 1):
            r0 = max(0, t * NQ - q0)
            r1 = min(128, (t + 1) * NQ - q0)
            qq0 = q0 + r0 - t * NQ
            logits[t, qq0 : qq0 + (r1 - r0), :] = o[
                r0:r1, base + (t - t0) * NW : base + (t - t0 + 1) * NW
            ]
    return logits


def kernel(query, support, scale, support_labels, n_way, n_shot):
    assert int(n_way) == NW and int(n_shot) * int(n_way) == NS
    assert query.shape == (B_TOT, NQ, D) and support.shape == (B_TOT, NS, D)
    nc = _get_nc()
    in_maps = _host_prep(support, query, support_labels, scale)
    res = run_bass_kernel_spmd(nc, in_maps, core_ids=list(range(N_CORES)))
    outs = []
    for core in range(N_CORES):
        o = np.asarray(res.results[core]["out"])      # [128, OUT_COLS]
        outs.append(_unshard_out(o))
    return np.ascontiguousarray(np.concatenate(outs, axis=0), dtype=np.float32)


# revision 20
# speedup vs baseline: 1.2072x; 1.2072x over previous
"""MetaOptNet SVM-CS head on 8 Trainium2 NeuronCores.

Math: the reference runs a 15-iteration Mehrotra interior-point solve of the
Crammer-Singer dual QP per task. Empirically (f64 replication) the IPM is
fully converged by iteration 15, so the target equals the QP optimum. We
compute that optimum with a fixed-matrix ADMM:

    per task:  K = S S^T  (25x25 Gram)
               W~ = rho * (K + (1+rho) I)^{-1}   (Newton-Schulz, 3 bf16 iters)
               10x ADMM (rho=8), in (d1 = u-y, oy = y+oh/rho) state form:
                   t = center_ways(W~ @ d1) + oy
                   d1' = min(t, 2h - t);  oy' = max(t - (h - oh/rho), oh/rho)
                   where h = (C + 1/rho) oh
               logits = Q @ (S^T x) * scale    (x = center_ways(W~ @ d1))

The equality constraint A z = 0 reduces to centering across ways because
A A^T = n_way I; the KKT matrix is way-block-diagonal with identical blocks
K + (1+rho)I, which is what makes one 25x25 inverse per task sufficient.

Sharding: pure data parallel, 16 tasks per core. Host-side work is layout
only (shard, transpose packing into 128-partition DMA tiles, one-hot
constants); all FLOPs run on-device.

PE-efficiency notes (the real HW bottleneck is LDWEIGHTS time, which the
CoreSim cost model does not model):
 - Gram: S^T shipped fp8-e3m4 (x64 prescale, descale folded into the mask
   const) in 32-col-strided task windows [128, 128] so one FWL weight load
   covers a 4-task group; the full-window matmul leaves junk in cross-task
   blocks which the mask zeroes before Newton-Schulz.
 - Stage 4 (w = S^T x): x is expanded to a block-diagonal [128, 20] tile
   (xdiag) so a single 128-col FWL load of S serves all 4 tasks of a group.
 - Stage 5 (logits = Q w): Q rides as stationary 128-col FWL windows over
   the task-concatenated query axis; the w vectors (5 cols/task) move.
   Logits come out in window-row layout [128, 125]; the host re-assembles.

Precision: QP in fp32 (ADMM) with bf16 Newton-Schulz; S/Q contracted in
bf16 except the Gram (fp8-e3m4). Measured end-to-end ~5e-3 relative
(tolerance 2e-2).
"""

import sys

sys.path.insert(0, "/opt/trn_rl_repo")

from contextlib import ExitStack

import numpy as np
import ml_dtypes

import concourse.bass as bass
import concourse.tile as tile
from concourse import mybir
from concourse.alu_op_type import AluOpType
from concourse.bass_utils import run_bass_kernel_spmd
from concourse.tile import TileContext

# ---------------------------------------------------------------------------
# Problem constants (hardcoded per the harness contract)
N_CORES = 8
B_TOT = 128
T = 16            # tasks per core
NS = 25           # support samples per task
NW = 5            # ways
NQ = 75           # queries per task
D = 2560          # feature dim
NCH = D // 128    # 20 d-chunks
G = 4             # task groups per core (4 tasks each -> 128-col windows)
GP = T // G       # tasks per group
RHO = 8.0
NS_C = 0.065      # Newton-Schulz init scale for H = K + 9I
NS_ITERS = 3
ADMM_ITERS = 5    # over-relaxed (alpha) ADMM needs half the plain iterations
ALPHA = 1.85      # over-relaxation factor, folded into Wt via the NS out-scale
C_REG = 0.1
GRAM_E3 = True    # ship S^T (Gram operand) as fp8-e3m4 (else bf16)
E3SCALE = 64.0 if GRAM_E3 else 1.0  # prescale; descaled via mask const
Q_E3 = True       # ship Q as fp8-e3m4 (x64, folded into the output scale)
QSCALE = 64.0 if Q_E3 else 1.0
NQT = 1280        # query cols padded to 10 x 128 windows
WIN = NQT // 128

F32 = mybir.dt.float32
BF16 = mybir.dt.bfloat16
FP8E3 = mybir.dt.float8e3 if GRAM_E3 else mybir.dt.bfloat16
QDT = mybir.dt.float8e3 if Q_E3 else mybir.dt.bfloat16


def _win_map():
    """Stage-5 window map: [(q0, q1, t0, t1, segs, colbase)] and total cols.
    segs: list of (g, ta, tb) group-contiguous task runs."""
    wins = []
    base = 0
    for w in range(WIN):
        q0 = w * 128
        q1 = min(q0 + 128, T * NQ)
        if q0 >= T * NQ:
            break
        t0, t1 = q0 // NQ, (q1 - 1) // NQ
        segs = []
        ta = t0
        while ta <= t1:
            g = ta // GP
            tb = min(t1, (g + 1) * GP - 1)
            segs.append((g, ta, tb))
            ta = tb + 1
        wins.append((q0, q1, t0, t1, segs, base))
        base += (t1 - t0 + 1) * NW
    return wins, base


WINDOWS, OUT_COLS = _win_map()


# ---------------------------------------------------------------------------
# The walrus build here encodes at most ONE sync-wait command per instruction
# (TPB_CTRL / S3_LW setupSyncWait raises "Too many sync wait commands").
# Tile's scheduler freely attaches several waits to one instruction, so after
# scheduling we split the excess onto NoOps inserted immediately before the
# instruction on the same engine — identical semantics, encodable waits.
def _split_waits(nc, max_waits=1):
    cnt = 0
    for blk in nc.m.functions[0].blocks:
        insns = blk.instructions
        idx = 0
        while idx < len(insns):
            ins = insns[idx]
            si = ins.sync_info
            waits = list(si.on_wait) if si and si.on_wait else []
            if len(waits) > max_waits:
                si.on_wait = waits[:max_waits]
                for w in waits[max_waits:]:
                    nop = mybir.InstNoOp(name=f"waitnop_{cnt}", ins=[], outs=[])
                    cnt += 1
                    nop.engine = ins.engine
                    nop.sync_info = mybir.SyncInfo(on_wait=[w], on_update=[])
                    nc.register_instruction(nop, overwrite=True)
                    insns.insert(idx, nop)
                    idx += 1
            idx += 1
    return cnt


# ---------------------------------------------------------------------------
def _build_program(repeat: int = 1, unroll: int = 1, variant: str = "full", upto: int = 5):
    """repeat>1 wraps the whole body in a hardware loop executing it that many
    times per launch — used by test.py to measure per-iteration device time as
    a slope, cancelling the (fixed, ~70ms) axon dispatch round-trip. The
    graded kernel() path always uses repeat=1, unroll=1, variant="full".

    unroll emits the body that many times inside the loop (separates loop
    overhead from body time). variant: "full" | "dma_only" (loads + store
    only) | "compute_only" (loads hoisted out of the loop)."""
    nc = bass.Bass("TRN2", target_bir_lowering=False)

    st_d = nc.dram_tensor("st", [NCH, 128, G * 128], FP8E3, kind="ExternalInput")
    sn_d = nc.dram_tensor("sn", [G, 128, D], BF16, kind="ExternalInput")
    qt_d = nc.dram_tensor("qt", [NCH, 128, NQT], QDT, kind="ExternalInput")
    ohc_d = nc.dram_tensor("ohc", [128, 20], F32, kind="ExternalInput")
    h2_d = nc.dram_tensor("h2", [128, 20], F32, kind="ExternalInput")
    hmo_d = nc.dram_tensor("hmo", [128, 20], F32, kind="ExternalInput")
    i2_d = nc.dram_tensor("i2", [128, 128], F32, kind="ExternalInput")
    cib_d = nc.dram_tensor("cib", [128, 128], BF16, kind="ExternalInput")
    nine_d = nc.dram_tensor("nine", [128, 128], F32, kind="ExternalInput")
    maskq_d = nc.dram_tensor("maskq", [128, 128], F32, kind="ExternalInput")
    scale_d = nc.dram_tensor("scale", [1, 1], F32, kind="ExternalInput")
    out_d = nc.dram_tensor("out", [128, OUT_COLS], F32, kind="ExternalOutput")
    if variant == "debug":
        dbg_h = nc.dram_tensor("dbg_h", [128, 128], F32, kind="ExternalOutput")
        dbg_wt = nc.dram_tensor("dbg_wt", [128, 128], F32, kind="ExternalOutput")
        dbg_xb = nc.dram_tensor("dbg_xb", [128, 20], BF16, kind="ExternalOutput")
        dbg_w = nc.dram_tensor("dbg_w", [128, NCH * GP * NW], BF16, kind="ExternalOutput")

    with ExitStack() as ctx:
        tc = ctx.enter_context(TileContext(nc))
        st_pool = ctx.enter_context(tc.tile_pool(name="st", bufs=1))
        sn_pool = ctx.enter_context(tc.tile_pool(name="sn", bufs=G))
        qt_pool = ctx.enter_context(tc.tile_pool(name="qt", bufs=NCH))
        consts = ctx.enter_context(tc.tile_pool(name="consts", bufs=1))
        mats = ctx.enter_context(tc.tile_pool(name="mats", bufs=12))
        state = ctx.enter_context(tc.tile_pool(name="state", bufs=14))
        wout = ctx.enter_context(tc.tile_pool(name="wout", bufs=4))

        def emit_loads(all_qt=False):
            # NS-critical consts first on the Act queue
            i2_sb = consts.tile([128, 128], F32, tag="i2")
            nc.scalar.dma_start(out=i2_sb, in_=i2_d[:, :])
            cib_sb = consts.tile([128, 128], BF16, tag="cib")
            nc.scalar.dma_start(out=cib_sb, in_=cib_d[:, :])
            nine_sb = consts.tile([128, 128], F32, tag="nine")
            nc.scalar.dma_start(out=nine_sb, in_=nine_d[:, :])
            maskq_sb = consts.tile([128, 128], F32, tag="maskq")
            nc.scalar.dma_start(out=maskq_sb, in_=maskq_d[:, :])
            st_tile = st_pool.tile([128, NCH * G * 128], FP8E3, tag="st")
            for j in range(4):
                nch4 = NCH // 4
                eng = nc.sync if j % 2 == 0 else nc.scalar
                eng.dma_start(
                    out=st_tile[:, j * nch4 * G * 128 : (j + 1) * nch4 * G * 128],
                    in_=st_d[j * nch4 : (j + 1) * nch4, :, :],
                )
            st_sb = [
                st_tile[:, c * G * 128 : (c + 1) * G * 128] for c in range(NCH)
            ]
            sn_sb = []
            for g in range(G):
                t_ = sn_pool.tile([128, D], BF16, tag="sn")
                nc.sync.dma_start(out=t_, in_=sn_d[g, :, :])
                sn_sb.append(t_)
            ohc_sb = consts.tile([128, 20], F32, tag="ohc")
            nc.scalar.dma_start(out=ohc_sb, in_=ohc_d[:, :])
            h2_sb = consts.tile([128, 20], F32, tag="h2")
            nc.scalar.dma_start(out=h2_sb, in_=h2_d[:, :])
            hmo_sb = consts.tile([128, 20], F32, tag="hmo")
            nc.scalar.dma_start(out=hmo_sb, in_=hmo_d[:, :])
            scale_sb = consts.tile([128, 1], F32, tag="scale")
            nc.scalar.dma_start(out=scale_sb, in_=scale_d[:, :].to_broadcast([128, 1]))

            # ADMM state: d1 = v - u + ohc (init ohc), s = h - u (init hmo)
            d1_sb = state.tile([128, 20], F32, tag="d1")
            nc.scalar.dma_start(out=d1_sb, in_=ohc_d[:, :])
            s0_sb = state.tile([128, 20], F32, tag="s0")
            nc.scalar.dma_start(out=s0_sb, in_=hmo_d[:, :])

            qt_sb = []
            n_pre = NCH if all_qt else NCH // 2
            for c in range(NCH):
                t_ = qt_pool.tile([128, NQT], QDT, tag="qt")
                if c < n_pre:
                    nc.scalar.dma_start(out=t_, in_=qt_d[c, :, :])
                qt_sb.append(t_)
            return dict(i2=i2_sb, cib=cib_sb, nine=nine_sb, maskq=maskq_sb,
                        st=st_sb, sn=sn_sb, ohc=ohc_sb, h2=h2_sb, hmo=hmo_sb,
                        scale=scale_sb, d1=d1_sb, s0=s0_sb, qt=qt_sb)

        def emit_compute(hd):
            def early_out():
                zt = consts.tile([128, OUT_COLS], F32, tag="outsb")
                nc.vector.memset(zt, 0.0)
                nc.sync.dma_start(out=out_d[:, :], in_=zt)
            st_sb, sn_sb, qt_sb = hd["st"], hd["sn"], hd["qt"]
            i2_sb, cib_sb, nine_sb, maskq_sb = (
                hd["i2"], hd["cib"], hd["nine"], hd["maskq"])
            ohc_sb, h2_sb, hmo_sb, scale_sb = (
                hd["ohc"], hd["h2"], hd["hmo"], hd["scale"])
            d1_sb = hd["d1"]

            # ---- stage 1: K = S S^T per 4-task 128-col window (fp8) -------
            # One FWL weight load per (group, chunk); cross-task junk blocks
            # are zeroed by maskq, which also folds in the 1/E3SCALE^2.
            hb_all = []
            with tc.tile_pool(name="kpsum", bufs=4, space="PSUM") as kpsum:
                kp_all = []
                for g in range(G):
                    kp = kpsum.tile([128, 128], F32, tag="kp")
                    for c in range(NCH):
                        nc.tensor.matmul(
                            kp,
                            lhsT=st_sb[c][:, g * 128 : (g + 1) * 128],
                            rhs=st_sb[c][:, g * 128 : (g + 1) * 128],
                            start=(c == 0),
                            stop=(c == NCH - 1),
                        )
                    kp_all.append(kp)
                h_all = []
                for g in range(G):
                    km = mats.tile([128, 128], F32, tag="km")
                    nc.vector.tensor_tensor(km, kp_all[g], maskq_sb, op=AluOpType.mult)
                    h_sb = mats.tile([128, 128], F32, tag="h")
                    nc.vector.tensor_tensor(h_sb, km, nine_sb, op=AluOpType.add)
                    h_all.append(h_sb)
                    hb = mats.tile([128, 128], BF16, tag="hb")
                    nc.vector.tensor_copy(hb, h_sb)
                    hb_all.append(hb)

            if upto < 2:
                return early_out()
            # ---- stage 2: Newton-Schulz inverse, all-bf16 -----------------
            # iters 0..n-2 in bf16 (NS self-corrects), final iter fp32 squares
            # the bf16 error away, so W~ is fp32-quality at reduced PE cost.
            wt_sb = []
            with tc.tile_pool(name="npsum", bufs=4, space="PSUM") as npsum:
                x_cur = [cib_sb] * G
                for it in range(NS_ITERS):
                    last = it == NS_ITERS - 1
                    prev_last = it == NS_ITERS - 2
                    for g in range(G):
                        t1p = npsum.tile([128, 128], F32, tag="t1p")
                        nc.tensor.matmul(
                            t1p,
                            lhsT=h_all[g] if last else hb_all[g],
                            rhs=x_cur[g],
                            start=True,
                            stop=True,
                        )
                        u_ns = mats.tile(
                            [128, 128], F32 if last else BF16,
                            tag="u_ns" if last else "u_nsb",
                        )
                        nc.vector.tensor_tensor(u_ns, i2_sb, t1p, op=AluOpType.subtract)
                        x2p = npsum.tile([128, 128], F32, tag="x2p")
                        nc.tensor.matmul(
                            x2p, lhsT=x_cur[g], rhs=u_ns, start=True, stop=True
                        )
                        if last:
                            wt = mats.tile([128, 128], F32, tag="wt")
                            nc.scalar.activation(
                                wt, x2p, mybir.ActivationFunctionType.Copy, scale=RHO * ALPHA
                            )
                            wt_sb.append(wt)
                        else:
                            x_next = mats.tile(
                                [128, 128], F32 if prev_last else BF16,
                                tag="x_ns" if prev_last else "x_nsb",
                            )
                            nc.scalar.activation(
                                x_next, x2p, mybir.ActivationFunctionType.Copy
                            )
                            x_cur[g] = x_next

            if upto < 3:
                return early_out()
            # ---- stage 3: over-relaxed ADMM -------------------------------
            # standard form on  min .5 z'Gz + e'z  st  Az=0, z<=h:
            #   z   = center(W (v - u + ohc));  zh = a*z + (1-a)*v
            #   r   = zh + u - h
            #   v'  = h + (r - |r|)/2;  u' = relu(r)
            #   d1' = v' - u' + ohc = (h + ohc) - |r|
            # with alpha folded into Wt (xp = a*W d1) and s = h - u as state.
            # Critical path per iter: reduce -> zn -> zh -> r -> |r| -> d1'
            # (6 DVE ops); v'/s' maintenance runs off-path, relu on ACT.
            xb_sb = None
            with ExitStack() as pctx:
                mpsum = pctx.enter_context(
                    tc.tile_pool(name="mpsum", bufs=2, space="PSUM"))
                v_sb = state.tile([128, 20], F32, tag="vst")
                nc.vector.memset(v_sb, 0.0)
                s_sb = hd["s0"]
                for it in range(ADMM_ITERS):
                    last = it == ADMM_ITERS - 1
                    xp = mpsum.tile([128, 20], F32, tag="mp")
                    for g in range(G):
                        nc.tensor.matmul(
                            xp[:, g * NW : (g + 1) * NW],
                            lhsT=wt_sb[g],
                            rhs=d1_sb[:, g * NW : (g + 1) * NW],
                            start=True,
                            stop=True,
                        )
                    msum = state.tile([128, 4], F32, tag="msum")
                    nc.vector.reduce_sum(
                        msum,
                        xp[:, :].rearrange("p (g w) -> p g w", w=NW),
                        axis=mybir.AxisListType.X,
                    )
                    msb = msum[:, :]
                    msb_ap = bass.AP(
                        tensor=msb.tensor, offset=msb.offset,
                        ap=[msb.ap[0], msb.ap[1], [0, NW]],
                    )
                    zn_sb = state.tile([128, 20], BF16 if last else F32,
                                       tag="xb" if last else "zn")
                    nc.vector.scalar_tensor_tensor(
                        out=zn_sb[:, :].rearrange("p (g w) -> p g w", w=NW),
                        in0=msb_ap,
                        scalar=-1.0 / NW,
                        in1=xp[:, :].rearrange("p (g w) -> p g w", w=NW),
                        op0=AluOpType.mult,
                        op1=AluOpType.add,
                    )
                    if last:
                        xb_sb = zn_sb
                        break
                    zh_sb = state.tile([128, 20], F32, tag="zh")
                    nc.vector.scalar_tensor_tensor(
                        out=zh_sb, in0=v_sb, scalar=(1.0 - ALPHA), in1=zn_sb,
                        op0=AluOpType.mult, op1=AluOpType.add,
                    )
                    r_sb = state.tile([128, 20], F32, tag="r")
                    nc.vector.tensor_tensor(r_sb, zh_sb, s_sb, op=AluOpType.subtract)
                    a_sb = state.tile([128, 20], F32, tag="absr")
                    nc.vector.scalar_tensor_tensor(
                        out=a_sb, in0=r_sb, scalar=-1.0, in1=r_sb,
                        op0=AluOpType.mult, op1=AluOpType.max,
                    )
                    d1_sb = state.tile([128, 20], F32, tag="d1n")
                    nc.vector.tensor_tensor(d1_sb, h2_sb, a_sb, op=AluOpType.subtract)
                    # off-critical-path state maintenance
                    rm_sb = state.tile([128, 20], F32, tag="rm")
                    nc.vector.tensor_tensor(rm_sb, r_sb, a_sb, op=AluOpType.subtract)
                    v_sb = state.tile([128, 20], F32, tag="vst2")
                    nc.vector.scalar_tensor_tensor(
                        out=v_sb, in0=rm_sb, scalar=0.5, in1=hmo_sb,
                        op0=AluOpType.mult, op1=AluOpType.add,
                    )
                    rl_sb = state.tile([128, 20], F32, tag="rl")
                    nc.scalar.activation(rl_sb, r_sb, mybir.ActivationFunctionType.Relu)
                    s_sb = state.tile([128, 20], F32, tag="sst")
                    nc.vector.tensor_tensor(s_sb, hmo_sb, rl_sb, op=AluOpType.subtract)

                if variant != "compute_only":
                    for c in range(NCH // 2, NCH):
                        nc.scalar.dma_start(out=qt_sb[c], in_=qt_d[c, :, :])

                if upto < 4:
                    return early_out()
                # ---- stage 4: w = S^T x via block-diagonal xdiag ----------
                # One [128,128] FWL load of S per (group, chunk) serves all
                # 4 tasks: xdiag has task tp's x block at rows tp*32+s,
                # cols tp*5+w, zero elsewhere.
                w_sb_g = []
                with tc.tile_pool(name="wpsum", bufs=4, space="PSUM") as wpsum:
                    for g in range(G):
                        xdiag = state.tile([128, 20], BF16, tag="xdiag")
                        nc.vector.memset(xdiag, 0.0)
                        for tp in range(GP):
                            sl = slice(tp * 32, tp * 32 + NS)
                            nc.vector.tensor_copy(
                                xdiag[sl, tp * NW : (tp + 1) * NW],
                                xb_sb[sl, g * NW : (g + 1) * NW],
                            )
                        wp = wpsum.tile([128, NCH * GP * NW], F32, tag="wp")
                        for c in range(NCH):
                            nc.tensor.matmul(
                                wp[:, c * GP * NW : (c + 1) * GP * NW],
                                lhsT=sn_sb[g][:, c * 128 : (c + 1) * 128],
                                rhs=xdiag,
                                start=True,
                                stop=True,
                            )
                        w_sb = wout.tile([128, NCH * GP * NW], BF16, tag="w")
                        nc.vector.tensor_copy(w_sb, wp)
                        w_sb_g.append(w_sb)

                if upto < 5:
                    return early_out()
                # ---- stage 5: logits via Q-stationary 128-col windows -----
                # lwin[w][r, 5j+v] = sum_d qt[d, w*128+r] * w_task(t0+j)[d, v]
                out_sb = consts.tile([128, OUT_COLS], F32, tag="outsb")
                with tc.tile_pool(name="lpsum", bufs=1, space="PSUM") as lpsum:
                    lp_all = lpsum.tile([128, OUT_COLS], F32, tag="lw")
                    lwin = []
                    for (q0, q1, t0, t1, segs, base) in WINDOWS:
                        lw_t = lp_all[:, base : base + (t1 - t0 + 1) * NW]
                        lwin.append(lw_t)
                    # window-outer / chunk-inner: PSUM accumulation groups
                    # must not interleave within a bank, so each window's
                    # 20-chunk accumulation completes before the next starts.
                    # matmul start=True clears has_written for the WHOLE psum
                    # bank (data intact), so each window's accumulation must
                    # finish before the next window's start, and only the
                    # first seg of c==0 may carry start=True (the other seg's
                    # region was just cleared, so its first write overwrites).
                    for wi, (q0, q1, t0, t1, segs, base) in enumerate(WINDOWS):
                        for c in range(NCH):
                            for si, (g, ta, tb) in enumerate(segs):
                                nc.tensor.matmul(
                                    lwin[wi][:, (ta - t0) * NW : (tb - t0 + 1) * NW],
                                    lhsT=qt_sb[c][:, q0 : q0 + 128],
                                    rhs=w_sb_g[g][
                                        :,
                                        c * GP * NW + (ta - g * GP) * NW :
                                        c * GP * NW + (tb - g * GP + 1) * NW,
                                    ],
                                    start=(c == 0 and si == 0),
                                    stop=(c == NCH - 1),
                                )
                    for wi, (q0, q1, t0, t1, segs, base) in enumerate(WINDOWS):
                        ncols = (t1 - t0 + 1) * NW
                        nc.scalar.activation(
                            out_sb[:, base : base + ncols],
                            lwin[wi],
                            mybir.ActivationFunctionType.Copy,
                            scale=scale_sb,
                        )
                nc.sync.dma_start(out=out_d[:, :], in_=out_sb)
                if variant == "debug":
                    nc.sync.dma_start(out=dbg_h[:, :], in_=h_all[0])
                    nc.sync.dma_start(out=dbg_wt[:, :], in_=wt_sb[0])
                    nc.sync.dma_start(out=dbg_xb[:, :], in_=xb_sb)
                    nc.sync.dma_start(out=dbg_w[:, :], in_=w_sb_g[0])

        def emit_body():
            if variant == "dma_only":
                emit_loads(all_qt=True)
                zt = consts.tile([128, OUT_COLS], F32, tag="outsb")
                nc.vector.memset(zt, 0.0)
                nc.sync.dma_start(out=out_d[:, :], in_=zt)
            else:
                hd = emit_loads()
                emit_compute(hd)

        if variant == "compute_only":
            hd0 = emit_loads(all_qt=True)
        if repeat > 1:
            ctx.enter_context(tc.For_i(0, repeat, 1))
        for _ in range(unroll):
            if variant == "compute_only":
                emit_compute(hd0)
            else:
                emit_body()

    _split_waits(nc)
    return nc


_NC_CACHE = None


def _get_nc():
    global _NC_CACHE
    if _NC_CACHE is None:
        _NC_CACHE = _build_program()
    return _NC_CACHE


# ---------------------------------------------------------------------------
def _host_prep(support, query, support_labels, scale):
    """Shard + pack into the DMA layouts. Layout only, no FLOPs."""
    f32 = np.float32
    bf = mybir.dt.np(BF16)
    e3 = mybir.dt.np(FP8E3)
    eye = np.eye(NS, dtype=f32)
    blockdiag = np.zeros((128, 128), dtype=f32)
    for tp in range(GP):
        blockdiag[tp * 32 : tp * 32 + NS, tp * 32 : tp * 32 + NS] = eye
    i2 = np.ascontiguousarray(2.0 * blockdiag, dtype=f32)
    ci = np.ascontiguousarray(NS_C * blockdiag, dtype=f32)
    nine = np.ascontiguousarray((1.0 + RHO) * blockdiag, dtype=f32)
    # 25x25 all-ones blocks at 32-spacing, folding in the e3m4 descale
    maskq = np.zeros((128, 128), dtype=f32)
    for tp in range(GP):
        for tq in range(GP):
            if tp == tq:
                maskq[tp * 32 : tp * 32 + NS, tp * 32 : tp * 32 + NS] = (
                    1.0 / (E3SCALE * E3SCALE)
                )
    sc = np.asarray(scale, dtype=f32).reshape(1, 1) / (QSCALE * ALPHA)

    in_maps = []
    for core in range(N_CORES):
        sl = slice(core * T, (core + 1) * T)
        S = np.asarray(support[sl], dtype=f32)        # [16,25,2560]
        Q = np.asarray(query[sl], dtype=f32)          # [16,75,2560]
        lab = np.asarray(support_labels[sl])          # [16,25] int
        # st: S^T x E3SCALE in 32-col strides: [NCH, 128, g*128 + tp*32 + s]
        s64 = np.clip(S * E3SCALE, -15.5, 15.5)
        stp = np.zeros((NCH, 128, G, GP, 32), dtype=f32)
        stp[:, :, :, :, :NS] = (
            s64.reshape(G, GP, NS, NCH, 128).transpose(3, 4, 0, 1, 2)
        )
        st = np.ascontiguousarray(
            stp.reshape(NCH, 128, G * 128).astype(e3)
        )
        sn = np.zeros((G, 128, D), dtype=bf)
        for tp in range(GP):
            sn[:, tp * 32 : tp * 32 + NS, :] = S.reshape(G, GP, NS, D)[:, tp].astype(bf)
        qtp = np.zeros((NCH, 128, NQT), dtype=f32)
        qtp[:, :, : T * NQ] = Q.transpose(2, 0, 1).reshape(NCH, 128, T * NQ)
        if Q_E3:
            qt = np.ascontiguousarray(
                np.clip(qtp * QSCALE, -15.5, 15.5).astype(mybir.dt.np(QDT))
            )
        else:
            qt = np.ascontiguousarray(qtp.astype(bf))
        oh = (lab[:, :, None] == np.arange(NW)[None, None, :]).astype(f32)
        # [16,25,5] -> [128,20]: row = tp*32+s, col = g*5+w
        ohm = np.zeros((128, 20), dtype=f32)
        ohr = oh.reshape(G, GP, NS, NW).transpose(1, 2, 0, 3).reshape(GP, NS, 20)
        for tp in range(GP):
            ohm[tp * 32 : tp * 32 + NS, :] = ohr[tp]
        in_maps.append(
            {
                "st": st,
                "sn": sn,
                "qt": qt,
                "ohc": np.ascontiguousarray(ohm / RHO),
                "h2": np.ascontiguousarray((C_REG + 1.0 / RHO) * ohm),
                "hmo": np.ascontiguousarray(C_REG * ohm),
                "i2": i2,
                "cib": np.ascontiguousarray(ci.astype(bf)),
                "nine": nine,
                "maskq": maskq,
                "scale": sc,
            }
        )
    return in_maps


def _unshard_out(o):
    """[128, OUT_COLS] window layout -> [T, NQ, NW]."""
    logits = np.zeros((T, NQ, NW), dtype=np.float32)
    for (q0, q1, t0, t1, segs, base) in WINDOWS:
        for t in range(t0, t1 + 1):
            r0 = max(0, t * NQ - q0)
            r1 = min(128, (t + 1) * NQ - q0)
            qq0 = q0 + r0 - t * NQ
            logits[t, qq0 : qq0 + (r1 - r0), :] = o[
                r0:r1, base + (t - t0) * NW : base + (t - t0 + 1) * NW
            ]
    return logits


def kernel(query, support, scale, support_labels, n_way, n_shot):
    assert int(n_way) == NW and int(n_shot) * int(n_way) == NS
    assert query.shape == (B_TOT, NQ, D) and support.shape == (B_TOT, NS, D)
    nc = _get_nc()
    in_maps = _host_prep(support, query, support_labels, scale)
    res = run_bass_kernel_spmd(nc, in_maps, core_ids=list(range(N_CORES)))
    outs = []
    for core in range(N_CORES):
        o = np.asarray(res.results[core]["out"])      # [128, OUT_COLS]
        outs.append(_unshard_out(o))
    return np.ascontiguousarray(np.concatenate(outs, axis=0), dtype=np.float32)


# revision 21
# speedup vs baseline: 1.2479x; 1.0337x over previous
"""MetaOptNet SVM-CS head on 8 Trainium2 NeuronCores.

Math: the reference runs a 15-iteration Mehrotra interior-point solve of the
Crammer-Singer dual QP per task. Empirically (f64 replication) the IPM is
fully converged by iteration 15, so the target equals the QP optimum. We
compute that optimum with a fixed-matrix ADMM:

    per task:  K = S S^T  (25x25 Gram)
               W~ = rho * (K + (1+rho) I)^{-1}   (Newton-Schulz, 3 bf16 iters)
               10x ADMM (rho=8), in (d1 = u-y, oy = y+oh/rho) state form:
                   t = center_ways(W~ @ d1) + oy
                   d1' = min(t, 2h - t);  oy' = max(t - (h - oh/rho), oh/rho)
                   where h = (C + 1/rho) oh
               logits = Q @ (S^T x) * scale    (x = center_ways(W~ @ d1))

The equality constraint A z = 0 reduces to centering across ways because
A A^T = n_way I; the KKT matrix is way-block-diagonal with identical blocks
K + (1+rho)I, which is what makes one 25x25 inverse per task sufficient.

Sharding: pure data parallel, 16 tasks per core. Host-side work is layout
only (shard, transpose packing into 128-partition DMA tiles, one-hot
constants); all FLOPs run on-device.

PE-efficiency notes (the real HW bottleneck is LDWEIGHTS time, which the
CoreSim cost model does not model):
 - Gram: S^T shipped fp8-e3m4 (x64 prescale, descale folded into the mask
   const) in 32-col-strided task windows [128, 128] so one FWL weight load
   covers a 4-task group; the full-window matmul leaves junk in cross-task
   blocks which the mask zeroes before Newton-Schulz.
 - Stage 4 (w = S^T x): x is expanded to a block-diagonal [128, 20] tile
   (xdiag) so a single 128-col FWL load of S serves all 4 tasks of a group.
 - Stage 5 (logits = Q w): Q rides as stationary 128-col FWL windows over
   the task-concatenated query axis; the w vectors (5 cols/task) move.
   Logits come out in window-row layout [128, 125]; the host re-assembles.

Precision: QP in fp32 (ADMM) with bf16 Newton-Schulz; S/Q contracted in
bf16 except the Gram (fp8-e3m4). Measured end-to-end ~5e-3 relative
(tolerance 2e-2).
"""

import sys

sys.path.insert(0, "/opt/trn_rl_repo")

from contextlib import ExitStack

import numpy as np
import ml_dtypes

import concourse.bass as bass
import concourse.tile as tile
from concourse import mybir
from concourse.alu_op_type import AluOpType
from concourse.bass_utils import run_bass_kernel_spmd
from concourse.tile import TileContext

# ---------------------------------------------------------------------------
# Problem constants (hardcoded per the harness contract)
N_CORES = 8
B_TOT = 128
T = 16            # tasks per core
NS = 25           # support samples per task
NW = 5            # ways
NQ = 75           # queries per task
D = 2560          # feature dim
NCH = D // 128    # 20 d-chunks
G = 4             # task groups per core (4 tasks each -> 128-col windows)
GP = T // G       # tasks per group
RHO = 8.0
NS_C = 0.065      # Newton-Schulz init scale for H = K + 9I
NS_ITERS = 3
ADMM_ITERS = 5    # over-relaxed (alpha) ADMM needs half the plain iterations
ALPHA = 1.85      # over-relaxation factor, folded into Wt via the NS out-scale
C_REG = 0.1
GRAM_E3 = True    # ship S^T (Gram operand) as fp8-e3m4 (else bf16)
E3SCALE = 64.0 if GRAM_E3 else 1.0  # prescale; descaled via mask const
Q_E3 = True       # ship Q as fp8-e3m4 (x64, folded into the output scale)
QSCALE = 64.0 if Q_E3 else 1.0
NQT = 1280        # query cols padded to 10 x 128 windows
WIN = NQT // 128

F32 = mybir.dt.float32
BF16 = mybir.dt.bfloat16
FP8E3 = mybir.dt.float8e3 if GRAM_E3 else mybir.dt.bfloat16
QDT = mybir.dt.float8e3 if Q_E3 else mybir.dt.bfloat16


def _win_map():
    """Stage-5 window map: [(q0, q1, t0, t1, segs, colbase)] and total cols.
    segs: list of (g, ta, tb) group-contiguous task runs."""
    wins = []
    base = 0
    for w in range(WIN):
        q0 = w * 128
        q1 = min(q0 + 128, T * NQ)
        if q0 >= T * NQ:
            break
        t0, t1 = q0 // NQ, (q1 - 1) // NQ
        segs = []
        ta = t0
        while ta <= t1:
            g = ta // GP
            tb = min(t1, (g + 1) * GP - 1)
            segs.append((g, ta, tb))
            ta = tb + 1
        wins.append((q0, q1, t0, t1, segs, base))
        base += (t1 - t0 + 1) * NW
    return wins, base


WINDOWS, OUT_COLS = _win_map()


# ---------------------------------------------------------------------------
# The walrus build here encodes at most ONE sync-wait command per instruction
# (TPB_CTRL / S3_LW setupSyncWait raises "Too many sync wait commands").
# Tile's scheduler freely attaches several waits to one instruction, so after
# scheduling we split the excess onto NoOps inserted immediately before the
# instruction on the same engine — identical semantics, encodable waits.
def _split_waits(nc, max_waits=1):
    cnt = 0
    for blk in nc.m.functions[0].blocks:
        insns = blk.instructions
        idx = 0
        while idx < len(insns):
            ins = insns[idx]
            si = ins.sync_info
            waits = list(si.on_wait) if si and si.on_wait else []
            if len(waits) > max_waits:
                si.on_wait = waits[:max_waits]
                for w in waits[max_waits:]:
                    nop = mybir.InstNoOp(name=f"waitnop_{cnt}", ins=[], outs=[])
                    cnt += 1
                    nop.engine = ins.engine
                    nop.sync_info = mybir.SyncInfo(on_wait=[w], on_update=[])
                    nc.register_instruction(nop, overwrite=True)
                    insns.insert(idx, nop)
                    idx += 1
            idx += 1
    return cnt


# ---------------------------------------------------------------------------
def _build_program(repeat: int = 1, unroll: int = 1, variant: str = "full", upto: int = 5):
    """repeat>1 wraps the whole body in a hardware loop executing it that many
    times per launch — used by test.py to measure per-iteration device time as
    a slope, cancelling the (fixed, ~70ms) axon dispatch round-trip. The
    graded kernel() path always uses repeat=1, unroll=1, variant="full".

    unroll emits the body that many times inside the loop (separates loop
    overhead from body time). variant: "full" | "dma_only" (loads + store
    only) | "compute_only" (loads hoisted out of the loop)."""
    nc = bass.Bass("TRN2", target_bir_lowering=False)

    st_d = nc.dram_tensor("st", [NCH, 128, G * 128], FP8E3, kind="ExternalInput")
    sn_d = nc.dram_tensor("sn", [G, 128, D], BF16, kind="ExternalInput")
    qt_d = nc.dram_tensor("qt", [NCH, 128, NQT], QDT, kind="ExternalInput")
    ohc_d = nc.dram_tensor("ohc", [128, 20], F32, kind="ExternalInput")
    h2_d = nc.dram_tensor("h2", [128, 20], F32, kind="ExternalInput")
    hmo_d = nc.dram_tensor("hmo", [128, 20], F32, kind="ExternalInput")
    i2_d = nc.dram_tensor("i2", [128, 128], F32, kind="ExternalInput")
    cib_d = nc.dram_tensor("cib", [128, 128], BF16, kind="ExternalInput")
    nine_d = nc.dram_tensor("nine", [128, 128], F32, kind="ExternalInput")
    maskq_d = nc.dram_tensor("maskq", [128, 128], F32, kind="ExternalInput")
    scale_d = nc.dram_tensor("scale", [1, 1], F32, kind="ExternalInput")
    out_d = nc.dram_tensor("out", [128, OUT_COLS], F32, kind="ExternalOutput")
    if variant == "debug":
        dbg_h = nc.dram_tensor("dbg_h", [128, 128], F32, kind="ExternalOutput")
        dbg_wt = nc.dram_tensor("dbg_wt", [128, 128], F32, kind="ExternalOutput")
        dbg_xb = nc.dram_tensor("dbg_xb", [128, 20], BF16, kind="ExternalOutput")
        dbg_w = nc.dram_tensor("dbg_w", [128, NCH * GP * NW], BF16, kind="ExternalOutput")

    with ExitStack() as ctx:
        tc = ctx.enter_context(TileContext(nc))
        st_pool = ctx.enter_context(tc.tile_pool(name="st", bufs=1))
        sn_pool = ctx.enter_context(tc.tile_pool(name="sn", bufs=G))
        qt_pool = ctx.enter_context(tc.tile_pool(name="qt", bufs=NCH))
        consts = ctx.enter_context(tc.tile_pool(name="consts", bufs=1))
        mats = ctx.enter_context(tc.tile_pool(name="mats", bufs=12))
        state = ctx.enter_context(tc.tile_pool(name="state", bufs=14))
        wout = ctx.enter_context(tc.tile_pool(name="wout", bufs=4))

        def emit_loads(all_qt=False):
            # NS-critical consts first on the Act queue
            i2_sb = consts.tile([128, 128], F32, tag="i2")
            nc.scalar.dma_start(out=i2_sb, in_=i2_d[:, :])
            cib_sb = consts.tile([128, 128], BF16, tag="cib")
            nc.scalar.dma_start(out=cib_sb, in_=cib_d[:, :])
            nine_sb = consts.tile([128, 128], F32, tag="nine")
            nc.scalar.dma_start(out=nine_sb, in_=nine_d[:, :])
            maskq_sb = consts.tile([128, 128], F32, tag="maskq")
            nc.scalar.dma_start(out=maskq_sb, in_=maskq_d[:, :])
            st_tile = st_pool.tile([128, NCH * G * 128], FP8E3, tag="st")
            for j in range(4):
                nch4 = NCH // 4
                eng = nc.sync if j % 2 == 0 else nc.scalar
                eng.dma_start(
                    out=st_tile[:, j * nch4 * G * 128 : (j + 1) * nch4 * G * 128],
                    in_=st_d[j * nch4 : (j + 1) * nch4, :, :],
                )
            st_sb = [
                st_tile[:, c * G * 128 : (c + 1) * G * 128] for c in range(NCH)
            ]
            sn_sb = []
            for g in range(G):
                t_ = sn_pool.tile([128, D], BF16, tag="sn")
                nc.sync.dma_start(out=t_, in_=sn_d[g, :, :])
                sn_sb.append(t_)
            ohc_sb = consts.tile([128, 20], F32, tag="ohc")
            nc.scalar.dma_start(out=ohc_sb, in_=ohc_d[:, :])
            h2_sb = consts.tile([128, 20], F32, tag="h2")
            nc.scalar.dma_start(out=h2_sb, in_=h2_d[:, :])
            hmo_sb = consts.tile([128, 20], F32, tag="hmo")
            nc.scalar.dma_start(out=hmo_sb, in_=hmo_d[:, :])
            scale_sb = consts.tile([128, 1], F32, tag="scale")
            nc.scalar.dma_start(out=scale_sb, in_=scale_d[:, :].to_broadcast([128, 1]))

            # ADMM state: d1 = v - u + ohc (init ohc), s = h - u (init hmo)
            d1_sb = state.tile([128, 20], F32, tag="d1")
            nc.scalar.dma_start(out=d1_sb, in_=ohc_d[:, :])
            s0_sb = state.tile([128, 20], F32, tag="s0")
            nc.scalar.dma_start(out=s0_sb, in_=hmo_d[:, :])

            qt_sb = []
            n_pre = NCH if all_qt else NCH // 2
            for c in range(NCH):
                t_ = qt_pool.tile([128, NQT], QDT, tag="qt")
                if c < n_pre:
                    nc.scalar.dma_start(out=t_, in_=qt_d[c, :, :])
                qt_sb.append(t_)
            return dict(i2=i2_sb, cib=cib_sb, nine=nine_sb, maskq=maskq_sb,
                        st=st_sb, sn=sn_sb, ohc=ohc_sb, h2=h2_sb, hmo=hmo_sb,
                        scale=scale_sb, d1=d1_sb, s0=s0_sb, qt=qt_sb)

        def emit_compute(hd):
            def early_out():
                zt = consts.tile([128, OUT_COLS], F32, tag="outsb")
                nc.vector.memset(zt, 0.0)
                nc.sync.dma_start(out=out_d[:, :], in_=zt)
            st_sb, sn_sb, qt_sb = hd["st"], hd["sn"], hd["qt"]
            i2_sb, cib_sb, nine_sb, maskq_sb = (
                hd["i2"], hd["cib"], hd["nine"], hd["maskq"])
            ohc_sb, h2_sb, hmo_sb, scale_sb = (
                hd["ohc"], hd["h2"], hd["hmo"], hd["scale"])
            d1_sb = hd["d1"]

            # ---- stage 1: K = S S^T per 4-task 128-col window (fp8) -------
            # One FWL weight load per (group, chunk); cross-task junk blocks
            # are zeroed by maskq, which also folds in the 1/E3SCALE^2.
            hb_all = []
            with tc.tile_pool(name="kpsum", bufs=4, space="PSUM") as kpsum:
                kp_all = []
                for g in range(G):
                    kp = kpsum.tile([128, 128], F32, tag="kp")
                    for c in range(NCH):
                        nc.tensor.matmul(
                            kp,
                            lhsT=st_sb[c][:, g * 128 : (g + 1) * 128],
                            rhs=st_sb[c][:, g * 128 : (g + 1) * 128],
                            start=(c == 0),
                            stop=(c == NCH - 1),
                        )
                    kp_all.append(kp)
                h_all = []
                for g in range(G):
                    km = mats.tile([128, 128], F32, tag="km")
                    nc.vector.tensor_tensor(km, kp_all[g], maskq_sb, op=AluOpType.mult)
                    h_sb = mats.tile([128, 128], F32, tag="h")
                    nc.vector.tensor_tensor(h_sb, km, nine_sb, op=AluOpType.add)
                    h_all.append(h_sb)
                    hb = mats.tile([128, 128], BF16, tag="hb")
                    nc.vector.tensor_copy(hb, h_sb)
                    hb_all.append(hb)

            if upto < 2:
                return early_out()
            # ---- stage 2: Newton-Schulz inverse, all-bf16 -----------------
            # iters 0..n-2 in bf16 (NS self-corrects), final iter fp32 squares
            # the bf16 error away, so W~ is fp32-quality at reduced PE cost.
            wt_sb = []
            with tc.tile_pool(name="npsum", bufs=4, space="PSUM") as npsum:
                x_cur = [cib_sb] * G
                for it in range(NS_ITERS):
                    last = it == NS_ITERS - 1
                    prev_last = it == NS_ITERS - 2
                    for g in range(G):
                        t1p = npsum.tile([128, 128], F32, tag="t1p")
                        nc.tensor.matmul(
                            t1p,
                            lhsT=h_all[g] if last else hb_all[g],
                            rhs=x_cur[g],
                            start=True,
                            stop=True,
                        )
                        u_ns = mats.tile(
                            [128, 128], F32 if last else BF16,
                            tag="u_ns" if last else "u_nsb",
                        )
                        nc.vector.tensor_tensor(u_ns, i2_sb, t1p, op=AluOpType.subtract)
                        x2p = npsum.tile([128, 128], F32, tag="x2p")
                        nc.tensor.matmul(
                            x2p, lhsT=x_cur[g], rhs=u_ns, start=True, stop=True
                        )
                        if last:
                            wt = mats.tile([128, 128], F32, tag="wt")
                            nc.scalar.activation(
                                wt, x2p, mybir.ActivationFunctionType.Copy, scale=RHO * ALPHA
                            )
                            wt_sb.append(wt)
                        else:
                            x_next = mats.tile(
                                [128, 128], F32 if prev_last else BF16,
                                tag="x_ns" if prev_last else "x_nsb",
                            )
                            nc.scalar.activation(
                                x_next, x2p, mybir.ActivationFunctionType.Copy
                            )
                            x_cur[g] = x_next

            if upto < 3:
                return early_out()
            # ---- stage 3: over-relaxed ADMM -------------------------------
            # standard form on  min .5 z'Gz + e'z  st  Az=0, z<=h:
            #   z   = center(W (v - u + ohc));  zh = a*z + (1-a)*v
            #   r   = zh + u - h
            #   v'  = h + (r - |r|)/2;  u' = relu(r)
            #   d1' = v' - u' + ohc = (h + ohc) - |r|
            # with alpha folded into Wt (xp = a*W d1) and s = h - u as state.
            # Critical path per iter: reduce -> zn -> zh -> r -> |r| -> d1'
            # (6 DVE ops); v'/s' maintenance runs off-path, relu on ACT.
            xb_sb = None
            with ExitStack() as pctx:
                mpsum = pctx.enter_context(
                    tc.tile_pool(name="mpsum", bufs=2, space="PSUM"))
                v_sb = state.tile([128, 20], F32, tag="vst")
                nc.vector.memset(v_sb, 0.0)
                s_sb = hd["s0"]
                for it in range(ADMM_ITERS):
                    last = it == ADMM_ITERS - 1
                    xp = mpsum.tile([128, 20], F32, tag="mp")
                    for g in range(G):
                        nc.tensor.matmul(
                            xp[:, g * NW : (g + 1) * NW],
                            lhsT=wt_sb[g],
                            rhs=d1_sb[:, g * NW : (g + 1) * NW],
                            start=True,
                            stop=True,
                        )
                    msum = state.tile([128, 4], F32, tag="msum")
                    nc.vector.reduce_sum(
                        msum,
                        xp[:, :].rearrange("p (g w) -> p g w", w=NW),
                        axis=mybir.AxisListType.X,
                    )
                    msb = msum[:, :]
                    msb_ap = bass.AP(
                        tensor=msb.tensor, offset=msb.offset,
                        ap=[msb.ap[0], msb.ap[1], [0, NW]],
                    )
                    zn_sb = state.tile([128, 20], BF16 if last else F32,
                                       tag="xb" if last else "zn")
                    nc.vector.scalar_tensor_tensor(
                        out=zn_sb[:, :].rearrange("p (g w) -> p g w", w=NW),
                        in0=msb_ap,
                        scalar=-1.0 / NW,
                        in1=xp[:, :].rearrange("p (g w) -> p g w", w=NW),
                        op0=AluOpType.mult,
                        op1=AluOpType.add,
                    )
                    if last:
                        xb_sb = zn_sb
                        break
                    zh_sb = state.tile([128, 20], F32, tag="zh")
                    nc.vector.scalar_tensor_tensor(
                        out=zh_sb, in0=v_sb, scalar=(1.0 - ALPHA), in1=zn_sb,
                        op0=AluOpType.mult, op1=AluOpType.add,
                    )
                    r_sb = state.tile([128, 20], F32, tag="r")
                    nc.vector.tensor_tensor(r_sb, zh_sb, s_sb, op=AluOpType.subtract)
                    a_sb = state.tile([128, 20], F32, tag="absr")
                    nc.vector.scalar_tensor_tensor(
                        out=a_sb, in0=r_sb, scalar=-1.0, in1=r_sb,
                        op0=AluOpType.mult, op1=AluOpType.max,
                    )
                    d1_sb = state.tile([128, 20], F32, tag="d1n")
                    nc.vector.tensor_tensor(d1_sb, h2_sb, a_sb, op=AluOpType.subtract)
                    # off-critical-path state maintenance
                    rm_sb = state.tile([128, 20], F32, tag="rm")
                    nc.vector.tensor_tensor(rm_sb, r_sb, a_sb, op=AluOpType.subtract)
                    v_sb = state.tile([128, 20], F32, tag="vst2")
                    nc.vector.scalar_tensor_tensor(
                        out=v_sb, in0=rm_sb, scalar=0.5, in1=hmo_sb,
                        op0=AluOpType.mult, op1=AluOpType.add,
                    )
                    rl_sb = state.tile([128, 20], F32, tag="rl")
                    nc.scalar.activation(rl_sb, r_sb, mybir.ActivationFunctionType.Relu)
                    s_sb = state.tile([128, 20], F32, tag="sst")
                    nc.vector.tensor_tensor(s_sb, hmo_sb, rl_sb, op=AluOpType.subtract)

                if variant != "compute_only":
                    for c in range(NCH // 2, NCH):
                        nc.scalar.dma_start(out=qt_sb[c], in_=qt_d[c, :, :])

                if upto < 4:
                    return early_out()
                # ---- stage 4: w = S^T x via block-diagonal xdiag ----------
                # One [128,128] FWL load of S per (group, chunk) serves all
                # 4 tasks: xdiag has task tp's x block at rows tp*32+s,
                # cols tp*5+w, zero elsewhere.
                w_sb_g = []
                with tc.tile_pool(name="wpsum", bufs=4, space="PSUM") as wpsum:
                    for g in range(G):
                        xdiag = state.tile([128, 20], BF16, tag="xdiag")
                        nc.vector.memset(xdiag, 0.0)
                        for tp in range(GP):
                            sl = slice(tp * 32, tp * 32 + NS)
                            nc.vector.tensor_copy(
                                xdiag[sl, tp * NW : (tp + 1) * NW],
                                xb_sb[sl, g * NW : (g + 1) * NW],
                            )
                        wp = wpsum.tile([128, NCH * GP * NW], F32, tag="wp")
                        for c in range(NCH):
                            nc.tensor.matmul(
                                wp[:, c * GP * NW : (c + 1) * GP * NW],
                                lhsT=sn_sb[g][:, c * 128 : (c + 1) * 128],
                                rhs=xdiag,
                                start=True,
                                stop=True,
                            )
                        w_sb = wout.tile([128, NCH * GP * NW], BF16, tag="w")
                        nc.vector.tensor_copy(w_sb, wp)
                        w_sb_g.append(w_sb)

                if upto < 5:
                    return early_out()
                # ---- stage 5: logits via Q-stationary 128-col windows -----
                # lwin[w][r, 5j+v] = sum_d qt[d, w*128+r] * w_task(t0+j)[d, v]
                out_sb = consts.tile([128, OUT_COLS], F32, tag="outsb")
                with tc.tile_pool(name="lpsum", bufs=1, space="PSUM") as lpsum:
                    lp_all = lpsum.tile([128, OUT_COLS], F32, tag="lw")
                    lwin = []
                    for (q0, q1, t0, t1, segs, base) in WINDOWS:
                        lw_t = lp_all[:, base : base + (t1 - t0 + 1) * NW]
                        lwin.append(lw_t)
                    # window-outer / chunk-inner: PSUM accumulation groups
                    # must not interleave within a bank, so each window's
                    # 20-chunk accumulation completes before the next starts.
                    # matmul start=True clears has_written for the WHOLE psum
                    # bank (data intact), so each window's accumulation must
                    # finish before the next window's start, and only the
                    # first seg of c==0 may carry start=True (the other seg's
                    # region was just cleared, so its first write overwrites).
                    for wi, (q0, q1, t0, t1, segs, base) in enumerate(WINDOWS):
                        for c in range(NCH):
                            for si, (g, ta, tb) in enumerate(segs):
                                nc.tensor.matmul(
                                    lwin[wi][:, (ta - t0) * NW : (tb - t0 + 1) * NW],
                                    lhsT=qt_sb[c][:, q0 : q0 + 128],
                                    rhs=w_sb_g[g][
                                        :,
                                        c * GP * NW + (ta - g * GP) * NW :
                                        c * GP * NW + (tb - g * GP + 1) * NW,
                                    ],
                                    start=(c == 0 and si == 0),
                                    stop=(c == NCH - 1),
                                )
                    for wi, (q0, q1, t0, t1, segs, base) in enumerate(WINDOWS):
                        ncols = (t1 - t0 + 1) * NW
                        nc.scalar.activation(
                            out_sb[:, base : base + ncols],
                            lwin[wi],
                            mybir.ActivationFunctionType.Copy,
                            scale=scale_sb,
                        )
                nc.sync.dma_start(out=out_d[:, :], in_=out_sb)
                if variant == "debug":
                    nc.sync.dma_start(out=dbg_h[:, :], in_=h_all[0])
                    nc.sync.dma_start(out=dbg_wt[:, :], in_=wt_sb[0])
                    nc.sync.dma_start(out=dbg_xb[:, :], in_=xb_sb)
                    nc.sync.dma_start(out=dbg_w[:, :], in_=w_sb_g[0])

        def emit_body():
            if variant == "dma_only":
                emit_loads(all_qt=True)
                zt = consts.tile([128, OUT_COLS], F32, tag="outsb")
                nc.vector.memset(zt, 0.0)
                nc.sync.dma_start(out=out_d[:, :], in_=zt)
            else:
                hd = emit_loads()
                emit_compute(hd)

        if variant == "compute_only":
            hd0 = emit_loads(all_qt=True)
        if repeat > 1:
            try:
                ctx.enter_context(tc.For_i(0, repeat, 1, staggered_reset=True))
            except Exception:
                ctx.enter_context(tc.For_i(0, repeat, 1))
        for _ in range(unroll):
            if variant == "compute_only":
                emit_compute(hd0)
            else:
                emit_body()

    _split_waits(nc)
    return nc


_NC_CACHE = None


def _get_nc():
    global _NC_CACHE
    if _NC_CACHE is None:
        _NC_CACHE = _build_program()
    return _NC_CACHE


# ---------------------------------------------------------------------------
def _host_prep(support, query, support_labels, scale):
    """Shard + pack into the DMA layouts. Layout only, no FLOPs."""
    f32 = np.float32
    bf = mybir.dt.np(BF16)
    e3 = mybir.dt.np(FP8E3)
    eye = np.eye(NS, dtype=f32)
    blockdiag = np.zeros((128, 128), dtype=f32)
    for tp in range(GP):
        blockdiag[tp * 32 : tp * 32 + NS, tp * 32 : tp * 32 + NS] = eye
    i2 = np.ascontiguousarray(2.0 * blockdiag, dtype=f32)
    ci = np.ascontiguousarray(NS_C * blockdiag, dtype=f32)
    nine = np.ascontiguousarray((1.0 + RHO) * blockdiag, dtype=f32)
    # 25x25 all-ones blocks at 32-spacing, folding in the e3m4 descale
    maskq = np.zeros((128, 128), dtype=f32)
    for tp in range(GP):
        for tq in range(GP):
            if tp == tq:
                maskq[tp * 32 : tp * 32 + NS, tp * 32 : tp * 32 + NS] = (
                    1.0 / (E3SCALE * E3SCALE)
                )
    sc = np.asarray(scale, dtype=f32).reshape(1, 1) / (QSCALE * ALPHA)

    in_maps = []
    for core in range(N_CORES):
        sl = slice(core * T, (core + 1) * T)
        S = np.asarray(support[sl], dtype=f32)        # [16,25,2560]
        Q = np.asarray(query[sl], dtype=f32)          # [16,75,2560]
        lab = np.asarray(support_labels[sl])          # [16,25] int
        # st: S^T x E3SCALE in 32-col strides: [NCH, 128, g*128 + tp*32 + s]
        s64 = np.clip(S * E3SCALE, -15.5, 15.5)
        stp = np.zeros((NCH, 128, G, GP, 32), dtype=f32)
        stp[:, :, :, :, :NS] = (
            s64.reshape(G, GP, NS, NCH, 128).transpose(3, 4, 0, 1, 2)
        )
        st = np.ascontiguousarray(
            stp.reshape(NCH, 128, G * 128).astype(e3)
        )
        sn = np.zeros((G, 128, D), dtype=bf)
        for tp in range(GP):
            sn[:, tp * 32 : tp * 32 + NS, :] = S.reshape(G, GP, NS, D)[:, tp].astype(bf)
        qtp = np.zeros((NCH, 128, NQT), dtype=f32)
        qtp[:, :, : T * NQ] = Q.transpose(2, 0, 1).reshape(NCH, 128, T * NQ)
        if Q_E3:
            qt = np.ascontiguousarray(
                np.clip(qtp * QSCALE, -15.5, 15.5).astype(mybir.dt.np(QDT))
            )
        else:
            qt = np.ascontiguousarray(qtp.astype(bf))
        oh = (lab[:, :, None] == np.arange(NW)[None, None, :]).astype(f32)
        # [16,25,5] -> [128,20]: row = tp*32+s, col = g*5+w
        ohm = np.zeros((128, 20), dtype=f32)
        ohr = oh.reshape(G, GP, NS, NW).transpose(1, 2, 0, 3).reshape(GP, NS, 20)
        for tp in range(GP):
            ohm[tp * 32 : tp * 32 + NS, :] = ohr[tp]
        in_maps.append(
            {
                "st": st,
                "sn": sn,
                "qt": qt,
                "ohc": np.ascontiguousarray(ohm / RHO),
                "h2": np.ascontiguousarray((C_REG + 1.0 / RHO) * ohm),
                "hmo": np.ascontiguousarray(C_REG * ohm),
                "i2": i2,
                "cib": np.ascontiguousarray(ci.astype(bf)),
                "nine": nine,
                "maskq": maskq,
                "scale": sc,
            }
        )
    return in_maps


def _unshard_out(o):
    """[128, OUT_COLS] window layout -> [T, NQ, NW]."""
    logits = np.zeros((T, NQ, NW), dtype=np.float32)
    for (q0, q1, t0, t1, segs, base) in WINDOWS:
        for t in range(t0, t1 + 1):
            r0 = max(0, t * NQ - q0)
            r1 = min(128, (t + 1) * NQ - q0)
            qq0 = q0 + r0 - t * NQ
            logits[t, qq0 : qq0 + (r1 - r0), :] = o[
                r0:r1, base + (t - t0) * NW : base + (t - t0 + 1) * NW
            ]
    return logits


def kernel(query, support, scale, support_labels, n_way, n_shot):
    assert int(n_way) == NW and int(n_shot) * int(n_way) == NS
    assert query.shape == (B_TOT, NQ, D) and support.shape == (B_TOT, NS, D)
    nc = _get_nc()
    in_maps = _host_prep(support, query, support_labels, scale)
    res = run_bass_kernel_spmd(nc, in_maps, core_ids=list(range(N_CORES)))
    outs = []
    for core in range(N_CORES):
        o = np.asarray(res.results[core]["out"])      # [128, OUT_COLS]
        outs.append(_unshard_out(o))
    return np.ascontiguousarray(np.concatenate(outs, axis=0), dtype=np.float32)


# revision 23
# speedup vs baseline: 1.3564x; 1.0869x over previous
"""MetaOptNet SVM-CS head on 8 Trainium2 NeuronCores.

Math: the reference runs a 15-iteration Mehrotra interior-point solve of the
Crammer-Singer dual QP per task. Empirically (f64 replication) the IPM is
fully converged by iteration 15, so the target equals the QP optimum. We
compute that optimum with a fixed-matrix ADMM:

    per task:  K = S S^T  (25x25 Gram)
               W~ = rho * (K + (1+rho) I)^{-1}   (Newton-Schulz, 3 bf16 iters)
               10x ADMM (rho=8), in (d1 = u-y, oy = y+oh/rho) state form:
                   t = center_ways(W~ @ d1) + oy
                   d1' = min(t, 2h - t);  oy' = max(t - (h - oh/rho), oh/rho)
                   where h = (C + 1/rho) oh
               logits = Q @ (S^T x) * scale    (x = center_ways(W~ @ d1))

The equality constraint A z = 0 reduces to centering across ways because
A A^T = n_way I; the KKT matrix is way-block-diagonal with identical blocks
K + (1+rho)I, which is what makes one 25x25 inverse per task sufficient.

Sharding: pure data parallel, 16 tasks per core. Host-side work is layout
only (shard, transpose packing into 128-partition DMA tiles, one-hot
constants); all FLOPs run on-device.

PE-efficiency notes (the real HW bottleneck is LDWEIGHTS time, which the
CoreSim cost model does not model):
 - Gram: S^T shipped fp8-e3m4 (x64 prescale, descale folded into the mask
   const) in 32-col-strided task windows [128, 128] so one FWL weight load
   covers a 4-task group; the full-window matmul leaves junk in cross-task
   blocks which the mask zeroes before Newton-Schulz.
 - Stage 4 (w = S^T x): x is expanded to a block-diagonal [128, 20] tile
   (xdiag) so a single 128-col FWL load of S serves all 4 tasks of a group.
 - Stage 5 (logits = Q w): Q rides as stationary 128-col FWL windows over
   the task-concatenated query axis; the w vectors (5 cols/task) move.
   Logits come out in window-row layout [128, 125]; the host re-assembles.

Precision: QP in fp32 (ADMM) with bf16 Newton-Schulz; S/Q contracted in
bf16 except the Gram (fp8-e3m4). Measured end-to-end ~5e-3 relative
(tolerance 2e-2).
"""

import sys

sys.path.insert(0, "/opt/trn_rl_repo")

from contextlib import ExitStack

import numpy as np
import ml_dtypes

import concourse.bass as bass
import concourse.tile as tile
from concourse import mybir
from concourse.alu_op_type import AluOpType
from concourse.bass_utils import run_bass_kernel_spmd
from concourse.tile import TileContext

# ---------------------------------------------------------------------------
# Problem constants (hardcoded per the harness contract)
N_CORES = 8
B_TOT = 128
T = 16            # tasks per core
NS = 25           # support samples per task
NW = 5            # ways
NQ = 75           # queries per task
D = 2560          # feature dim
NCH = D // 128    # 20 d-chunks
G = 4             # task groups per core (4 tasks each -> 128-col windows)
GP = T // G       # tasks per group
RHO = 8.0
NS_C = 0.065      # Newton-Schulz init scale for H = K + 9I
NS_ITERS = 3
ADMM_ITERS = 4    # over-relaxed (alpha) ADMM converges ~2.5x faster than plain
ALPHA = 1.7       # over-relaxation factor, folded into Wt via the NS out-scale
C_REG = 0.1
GRAM_E3 = True    # ship S^T (Gram operand) as fp8-e3m4 (else bf16)
E3SCALE = 64.0 if GRAM_E3 else 1.0  # prescale; descaled via mask const
Q_E3 = True       # ship Q as fp8-e3m4 (x64, folded into the output scale)
QSCALE = 64.0 if Q_E3 else 1.0
NQT = 1280        # query cols padded to 10 x 128 windows
WIN = NQT // 128

F32 = mybir.dt.float32
BF16 = mybir.dt.bfloat16
FP8E3 = mybir.dt.float8e3 if GRAM_E3 else mybir.dt.bfloat16
QDT = mybir.dt.float8e3 if Q_E3 else mybir.dt.bfloat16


def _win_map():
    """Stage-5 window map: [(q0, q1, t0, t1, segs, colbase)] and total cols.
    segs: list of (g, ta, tb) group-contiguous task runs."""
    wins = []
    base = 0
    for w in range(WIN):
        q0 = w * 128
        q1 = min(q0 + 128, T * NQ)
        if q0 >= T * NQ:
            break
        t0, t1 = q0 // NQ, (q1 - 1) // NQ
        segs = []
        ta = t0
        while ta <= t1:
            g = ta // GP
            tb = min(t1, (g + 1) * GP - 1)
            segs.append((g, ta, tb))
            ta = tb + 1
        wins.append((q0, q1, t0, t1, segs, base))
        base += (t1 - t0 + 1) * NW
    return wins, base


WINDOWS, OUT_COLS = _win_map()


# ---------------------------------------------------------------------------
# The walrus build here encodes at most ONE sync-wait command per instruction
# (TPB_CTRL / S3_LW setupSyncWait raises "Too many sync wait commands").
# Tile's scheduler freely attaches several waits to one instruction, so after
# scheduling we split the excess onto NoOps inserted immediately before the
# instruction on the same engine — identical semantics, encodable waits.
def _split_waits(nc, max_waits=1):
    cnt = 0
    for blk in nc.m.functions[0].blocks:
        insns = blk.instructions
        idx = 0
        while idx < len(insns):
            ins = insns[idx]
            si = ins.sync_info
            waits = list(si.on_wait) if si and si.on_wait else []
            if len(waits) > max_waits:
                si.on_wait = waits[:max_waits]
                for w in waits[max_waits:]:
                    nop = mybir.InstNoOp(name=f"waitnop_{cnt}", ins=[], outs=[])
                    cnt += 1
                    nop.engine = ins.engine
                    nop.sync_info = mybir.SyncInfo(on_wait=[w], on_update=[])
                    nc.register_instruction(nop, overwrite=True)
                    insns.insert(idx, nop)
                    idx += 1
            idx += 1
    return cnt


# ---------------------------------------------------------------------------
def _build_program(repeat: int = 1, unroll: int = 1, variant: str = "full", upto: int = 5):
    """repeat>1 wraps the whole body in a hardware loop executing it that many
    times per launch — used by test.py to measure per-iteration device time as
    a slope, cancelling the (fixed, ~70ms) axon dispatch round-trip. The
    graded kernel() path always uses repeat=1, unroll=1, variant="full".

    unroll emits the body that many times inside the loop (separates loop
    overhead from body time). variant: "full" | "dma_only" (loads + store
    only) | "compute_only" (loads hoisted out of the loop)."""
    nc = bass.Bass("TRN2", target_bir_lowering=False)

    st_d = nc.dram_tensor("st", [NCH, 128, G * 128], FP8E3, kind="ExternalInput")
    sn_d = nc.dram_tensor("sn", [G, 128, D], BF16, kind="ExternalInput")
    qt_d = nc.dram_tensor("qt", [NCH, 128, NQT], QDT, kind="ExternalInput")
    ohc_d = nc.dram_tensor("ohc", [128, 20], F32, kind="ExternalInput")
    h2_d = nc.dram_tensor("h2", [128, 20], F32, kind="ExternalInput")
    hmo_d = nc.dram_tensor("hmo", [128, 20], F32, kind="ExternalInput")
    i2_d = nc.dram_tensor("i2", [128, 128], F32, kind="ExternalInput")
    cib_d = nc.dram_tensor("cib", [128, 128], BF16, kind="ExternalInput")
    nine_d = nc.dram_tensor("nine", [128, 128], F32, kind="ExternalInput")
    maskq_d = nc.dram_tensor("maskq", [128, 128], F32, kind="ExternalInput")
    scale_d = nc.dram_tensor("scale", [1, 1], F32, kind="ExternalInput")
    out_d = nc.dram_tensor("out", [128, OUT_COLS], F32, kind="ExternalOutput")
    if variant == "debug":
        dbg_h = nc.dram_tensor("dbg_h", [128, 128], F32, kind="ExternalOutput")
        dbg_wt = nc.dram_tensor("dbg_wt", [128, 128], F32, kind="ExternalOutput")
        dbg_xb = nc.dram_tensor("dbg_xb", [128, 20], BF16, kind="ExternalOutput")
        dbg_w = nc.dram_tensor("dbg_w", [128, NCH * GP * NW], BF16, kind="ExternalOutput")

    with ExitStack() as ctx:
        tc = ctx.enter_context(TileContext(nc))
        st_pool = ctx.enter_context(tc.tile_pool(name="st", bufs=1))
        sn_pool = ctx.enter_context(tc.tile_pool(name="sn", bufs=G))
        qt_pool = ctx.enter_context(tc.tile_pool(name="qt", bufs=NCH))
        consts = ctx.enter_context(tc.tile_pool(name="consts", bufs=1))
        mats = ctx.enter_context(tc.tile_pool(name="mats", bufs=12))
        state = ctx.enter_context(tc.tile_pool(name="state", bufs=14))
        wout = ctx.enter_context(tc.tile_pool(name="wout", bufs=4))

        def emit_loads(all_qt=False):
            # NS-critical consts first on the Act queue
            i2_sb = consts.tile([128, 128], F32, tag="i2")
            nc.scalar.dma_start(out=i2_sb, in_=i2_d[:, :])
            cib_sb = consts.tile([128, 128], BF16, tag="cib")
            nc.scalar.dma_start(out=cib_sb, in_=cib_d[:, :])
            nine_sb = consts.tile([128, 128], F32, tag="nine")
            nc.scalar.dma_start(out=nine_sb, in_=nine_d[:, :])
            maskq_sb = consts.tile([128, 128], F32, tag="maskq")
            nc.scalar.dma_start(out=maskq_sb, in_=maskq_d[:, :])
            st_tile = st_pool.tile([128, NCH * G * 128], FP8E3, tag="st")
            for j in range(4):
                nch4 = NCH // 4
                eng = nc.sync if j % 2 == 0 else nc.scalar
                eng.dma_start(
                    out=st_tile[:, j * nch4 * G * 128 : (j + 1) * nch4 * G * 128],
                    in_=st_d[j * nch4 : (j + 1) * nch4, :, :],
                )
            st_sb = [
                st_tile[:, c * G * 128 : (c + 1) * G * 128] for c in range(NCH)
            ]
            sn_sb = []
            for g in range(G):
                t_ = sn_pool.tile([128, D], BF16, tag="sn")
                nc.sync.dma_start(out=t_, in_=sn_d[g, :, :])
                sn_sb.append(t_)
            ohc_sb = consts.tile([128, 20], F32, tag="ohc")
            nc.scalar.dma_start(out=ohc_sb, in_=ohc_d[:, :])
            h2_sb = consts.tile([128, 20], F32, tag="h2")
            nc.scalar.dma_start(out=h2_sb, in_=h2_d[:, :])
            hmo_sb = consts.tile([128, 20], F32, tag="hmo")
            nc.scalar.dma_start(out=hmo_sb, in_=hmo_d[:, :])
            scale_sb = consts.tile([128, 1], F32, tag="scale")
            nc.scalar.dma_start(out=scale_sb, in_=scale_d[:, :].to_broadcast([128, 1]))

            # ADMM state: d1 = v - u + ohc (init ohc), s = h - u (init hmo)
            d1_sb = state.tile([128, 20], F32, tag="d1")
            nc.scalar.dma_start(out=d1_sb, in_=ohc_d[:, :])
            s0_sb = state.tile([128, 20], F32, tag="s0")
            nc.scalar.dma_start(out=s0_sb, in_=hmo_d[:, :])

            qt_sb = []
            n_pre = NCH if all_qt else NCH // 2
            for c in range(NCH):
                t_ = qt_pool.tile([128, NQT], QDT, tag="qt")
                if c < n_pre:
                    nc.scalar.dma_start(out=t_, in_=qt_d[c, :, :])
                qt_sb.append(t_)
            return dict(i2=i2_sb, cib=cib_sb, nine=nine_sb, maskq=maskq_sb,
                        st=st_sb, sn=sn_sb, ohc=ohc_sb, h2=h2_sb, hmo=hmo_sb,
                        scale=scale_sb, d1=d1_sb, s0=s0_sb, qt=qt_sb)

        def emit_compute(hd):
            def early_out():
                zt = consts.tile([128, OUT_COLS], F32, tag="outsb")
                nc.vector.memset(zt, 0.0)
                nc.sync.dma_start(out=out_d[:, :], in_=zt)
            st_sb, sn_sb, qt_sb = hd["st"], hd["sn"], hd["qt"]
            i2_sb, cib_sb, nine_sb, maskq_sb = (
                hd["i2"], hd["cib"], hd["nine"], hd["maskq"])
            ohc_sb, h2_sb, hmo_sb, scale_sb = (
                hd["ohc"], hd["h2"], hd["hmo"], hd["scale"])
            d1_sb = hd["d1"]

            # ---- stage 1: K = S S^T per 4-task 128-col window (fp8) -------
            # One FWL weight load per (group, chunk); cross-task junk blocks
            # are zeroed by maskq, which also folds in the 1/E3SCALE^2.
            hb_all = []
            with tc.tile_pool(name="kpsum", bufs=4, space="PSUM") as kpsum:
                kp_all = []
                for g in range(G):
                    kp = kpsum.tile([128, 128], F32, tag="kp")
                    for c in range(NCH):
                        nc.tensor.matmul(
                            kp,
                            lhsT=st_sb[c][:, g * 128 : (g + 1) * 128],
                            rhs=st_sb[c][:, g * 128 : (g + 1) * 128],
                            start=(c == 0),
                            stop=(c == NCH - 1),
                        )
                    kp_all.append(kp)
                h_all = []
                for g in range(G):
                    km = mats.tile([128, 128], F32, tag="km")
                    nc.vector.tensor_tensor(km, kp_all[g], maskq_sb, op=AluOpType.mult)
                    h_sb = mats.tile([128, 128], F32, tag="h")
                    nc.vector.tensor_tensor(h_sb, km, nine_sb, op=AluOpType.add)
                    h_all.append(h_sb)
                    hb = mats.tile([128, 128], BF16, tag="hb")
                    nc.vector.tensor_copy(hb, h_sb)
                    hb_all.append(hb)

            if upto < 2:
                return early_out()
            # ---- stage 2: Newton-Schulz inverse, all-bf16 -----------------
            # iters 0..n-2 in bf16 (NS self-corrects), final iter fp32 squares
            # the bf16 error away, so W~ is fp32-quality at reduced PE cost.
            wt_sb = []
            with tc.tile_pool(name="npsum", bufs=4, space="PSUM") as npsum:
                x_cur = [cib_sb] * G
                for it in range(NS_ITERS):
                    last = it == NS_ITERS - 1
                    prev_last = it == NS_ITERS - 2
                    for g in range(G):
                        t1p = npsum.tile([128, 128], F32, tag="t1p")
                        nc.tensor.matmul(
                            t1p,
                            lhsT=h_all[g] if last else hb_all[g],
                            rhs=x_cur[g],
                            start=True,
                            stop=True,
                        )
                        u_ns = mats.tile(
                            [128, 128], F32 if last else BF16,
                            tag="u_ns" if last else "u_nsb",
                        )
                        nc.vector.tensor_tensor(u_ns, i2_sb, t1p, op=AluOpType.subtract)
                        x2p = npsum.tile([128, 128], F32, tag="x2p")
                        nc.tensor.matmul(
                            x2p, lhsT=x_cur[g], rhs=u_ns, start=True, stop=True
                        )
                        if last:
                            wt = mats.tile([128, 128], F32, tag="wt")
                            nc.scalar.activation(
                                wt, x2p, mybir.ActivationFunctionType.Copy, scale=RHO * ALPHA
                            )
                            wt_sb.append(wt)
                        else:
                            x_next = mats.tile(
                                [128, 128], F32 if prev_last else BF16,
                                tag="x_ns" if prev_last else "x_nsb",
                            )
                            nc.scalar.activation(
                                x_next, x2p, mybir.ActivationFunctionType.Copy
                            )
                            x_cur[g] = x_next

            if upto < 3:
                return early_out()
            # ---- stage 3: over-relaxed ADMM -------------------------------
            # standard form on  min .5 z'Gz + e'z  st  Az=0, z<=h:
            #   z   = center(W (v - u + ohc));  zh = a*z + (1-a)*v
            #   r   = zh + u - h
            #   v'  = h + (r - |r|)/2;  u' = relu(r)
            #   d1' = v' - u' + ohc = (h + ohc) - |r|
            # with alpha folded into Wt (xp = a*W d1) and s = h - u as state.
            # Critical path per iter: reduce -> zn -> zh -> r -> |r| -> d1'
            # (6 DVE ops); v'/s' maintenance runs off-path, relu on ACT.
            xb_sb = None
            with ExitStack() as pctx:
                mpsum = pctx.enter_context(
                    tc.tile_pool(name="mpsum", bufs=2, space="PSUM"))
                v_sb = state.tile([128, 20], F32, tag="vst")
                nc.vector.memset(v_sb, 0.0)
                s_sb = hd["s0"]
                for it in range(ADMM_ITERS):
                    last = it == ADMM_ITERS - 1
                    xp = mpsum.tile([128, 20], F32, tag="mp")
                    for g in range(G):
                        nc.tensor.matmul(
                            xp[:, g * NW : (g + 1) * NW],
                            lhsT=wt_sb[g],
                            rhs=d1_sb[:, g * NW : (g + 1) * NW],
                            start=True,
                            stop=True,
                        )
                    msum = state.tile([128, 4], F32, tag="msum")
                    nc.vector.reduce_sum(
                        msum,
                        xp[:, :].rearrange("p (g w) -> p g w", w=NW),
                        axis=mybir.AxisListType.X,
                    )
                    msb = msum[:, :]
                    msb_ap = bass.AP(
                        tensor=msb.tensor, offset=msb.offset,
                        ap=[msb.ap[0], msb.ap[1], [0, NW]],
                    )
                    zn_sb = state.tile([128, 20], BF16 if last else F32,
                                       tag="xb" if last else "zn")
                    nc.vector.scalar_tensor_tensor(
                        out=zn_sb[:, :].rearrange("p (g w) -> p g w", w=NW),
                        in0=msb_ap,
                        scalar=-1.0 / NW,
                        in1=xp[:, :].rearrange("p (g w) -> p g w", w=NW),
                        op0=AluOpType.mult,
                        op1=AluOpType.add,
                    )
                    if last:
                        xb_sb = zn_sb
                        break
                    zh_sb = state.tile([128, 20], F32, tag="zh")
                    nc.vector.scalar_tensor_tensor(
                        out=zh_sb, in0=v_sb, scalar=(1.0 - ALPHA), in1=zn_sb,
                        op0=AluOpType.mult, op1=AluOpType.add,
                    )
                    r_sb = state.tile([128, 20], F32, tag="r")
                    nc.vector.tensor_tensor(r_sb, zh_sb, s_sb, op=AluOpType.subtract)
                    a_sb = state.tile([128, 20], F32, tag="absr")
                    nc.vector.scalar_tensor_tensor(
                        out=a_sb, in0=r_sb, scalar=-1.0, in1=r_sb,
                        op0=AluOpType.mult, op1=AluOpType.max,
                    )
                    d1_sb = state.tile([128, 20], F32, tag="d1n")
                    nc.vector.tensor_tensor(d1_sb, h2_sb, a_sb, op=AluOpType.subtract)
                    # off-critical-path state maintenance
                    rm_sb = state.tile([128, 20], F32, tag="rm")
                    nc.vector.tensor_tensor(rm_sb, r_sb, a_sb, op=AluOpType.subtract)
                    v_sb = state.tile([128, 20], F32, tag="vst2")
                    nc.vector.scalar_tensor_tensor(
                        out=v_sb, in0=rm_sb, scalar=0.5, in1=hmo_sb,
                        op0=AluOpType.mult, op1=AluOpType.add,
                    )
                    rl_sb = state.tile([128, 20], F32, tag="rl")
                    nc.scalar.activation(rl_sb, r_sb, mybir.ActivationFunctionType.Relu)
                    s_sb = state.tile([128, 20], F32, tag="sst")
                    nc.vector.tensor_tensor(s_sb, hmo_sb, rl_sb, op=AluOpType.subtract)

                if variant != "compute_only":
                    for c in range(NCH // 2, NCH):
                        nc.scalar.dma_start(out=qt_sb[c], in_=qt_d[c, :, :])

                if upto < 4:
                    return early_out()
                # ---- stage 4: w = S^T x via block-diagonal xdiag ----------
                # One [128,128] FWL load of S per (group, chunk) serves all
                # 4 tasks: xdiag has task tp's x block at rows tp*32+s,
                # cols tp*5+w, zero elsewhere.
                # w_all holds w in global (chunk, task, way) column order so
                # stage 5 reads one contiguous slice per (window, chunk).
                w_all = wout.tile([128, NCH * T * NW], BF16, tag="w")
                with tc.tile_pool(name="wpsum", bufs=4, space="PSUM") as wpsum:
                    for g in range(G):
                        xdiag = state.tile([128, 20], BF16, tag="xdiag")
                        nc.vector.memset(xdiag, 0.0)
                        for tp in range(GP):
                            sl = slice(tp * 32, tp * 32 + NS)
                            nc.vector.tensor_copy(
                                xdiag[sl, tp * NW : (tp + 1) * NW],
                                xb_sb[sl, g * NW : (g + 1) * NW],
                            )
                        wp = wpsum.tile([128, NCH * GP * NW], F32, tag="wp")
                        for c in range(NCH):
                            nc.tensor.matmul(
                                wp[:, c * GP * NW : (c + 1) * GP * NW],
                                lhsT=sn_sb[g][:, c * 128 : (c + 1) * 128],
                                rhs=xdiag,
                                start=True,
                                stop=True,
                            )
                        wa = w_all[:, :]
                        out_ap = bass.AP(
                            tensor=wa.tensor,
                            offset=wa.offset + g * GP * NW,
                            ap=[wa.ap[0], [T * NW, NCH], [1, GP * NW]],
                        )
                        nc.vector.tensor_copy(
                            out_ap,
                            wp[:, :].rearrange("p (c k) -> p c k", k=GP * NW),
                        )

                if upto < 5:
                    return early_out()
                # ---- stage 5: logits via Q-stationary 128-col windows -----
                # lwin[w][r, 5j+v] = sum_d qt[d, w*128+r] * w_task(t0+j)[d, v]
                out_sb = consts.tile([128, OUT_COLS], F32, tag="outsb")
                with tc.tile_pool(name="lpsum", bufs=1, space="PSUM") as lpsum:
                    lp_all = lpsum.tile([128, OUT_COLS], F32, tag="lw")
                    lwin = []
                    for (q0, q1, t0, t1, segs, base) in WINDOWS:
                        lw_t = lp_all[:, base : base + (t1 - t0 + 1) * NW]
                        lwin.append(lw_t)
                    # window-outer / chunk-inner: PSUM accumulation groups
                    # must not interleave within a bank, so each window's
                    # 20-chunk accumulation completes before the next starts.
                    # matmul start=True clears has_written for the WHOLE psum
                    # bank (data intact), so each window's accumulation must
                    # finish before the next window's start.
                    for wi, (q0, q1, t0, t1, segs, base) in enumerate(WINDOWS):
                        for c in range(NCH):
                            nc.tensor.matmul(
                                lwin[wi],
                                lhsT=qt_sb[c][:, q0 : q0 + 128],
                                rhs=w_all[
                                    :,
                                    c * T * NW + t0 * NW :
                                    c * T * NW + (t1 + 1) * NW,
                                ],
                                start=(c == 0),
                                stop=(c == NCH - 1),
                            )
                    for wi, (q0, q1, t0, t1, segs, base) in enumerate(WINDOWS):
                        ncols = (t1 - t0 + 1) * NW
                        nc.scalar.activation(
                            out_sb[:, base : base + ncols],
                            lwin[wi],
                            mybir.ActivationFunctionType.Copy,
                            scale=scale_sb,
                        )
                nc.sync.dma_start(out=out_d[:, :], in_=out_sb)
                if variant == "debug":
                    nc.sync.dma_start(out=dbg_h[:, :], in_=h_all[0])
                    nc.sync.dma_start(out=dbg_wt[:, :], in_=wt_sb[0])
                    nc.sync.dma_start(out=dbg_xb[:, :], in_=xb_sb)
                    nc.sync.dma_start(out=dbg_w[:, :], in_=w_all[:, : NCH * GP * NW])

        def emit_body():
            if variant == "dma_only":
                emit_loads(all_qt=True)
                zt = consts.tile([128, OUT_COLS], F32, tag="outsb")
                nc.vector.memset(zt, 0.0)
                nc.sync.dma_start(out=out_d[:, :], in_=zt)
            else:
                hd = emit_loads()
                emit_compute(hd)

        if variant == "compute_only":
            hd0 = emit_loads(all_qt=True)
        if repeat > 1:
            try:
                ctx.enter_context(tc.For_i(0, repeat, 1, staggered_reset=True))
            except Exception:
                ctx.enter_context(tc.For_i(0, repeat, 1))
        for _ in range(unroll):
            if variant == "compute_only":
                emit_compute(hd0)
            else:
                emit_body()

    _split_waits(nc)
    return nc


_NC_CACHE = None


def _get_nc():
    global _NC_CACHE
    if _NC_CACHE is None:
        _NC_CACHE = _build_program()
    return _NC_CACHE


# ---------------------------------------------------------------------------
def _host_prep(support, query, support_labels, scale):
    """Shard + pack into the DMA layouts. Layout only, no FLOPs."""
    f32 = np.float32
    bf = mybir.dt.np(BF16)
    e3 = mybir.dt.np(FP8E3)
    eye = np.eye(NS, dtype=f32)
    blockdiag = np.zeros((128, 128), dtype=f32)
    for tp in range(GP):
        blockdiag[tp * 32 : tp * 32 + NS, tp * 32 : tp * 32 + NS] = eye
    i2 = np.ascontiguousarray(2.0 * blockdiag, dtype=f32)
    ci = np.ascontiguousarray(NS_C * blockdiag, dtype=f32)
    nine = np.ascontiguousarray((1.0 + RHO) * blockdiag, dtype=f32)
    # 25x25 all-ones blocks at 32-spacing, folding in the e3m4 descale
    maskq = np.zeros((128, 128), dtype=f32)
    for tp in range(GP):
        for tq in range(GP):
            if tp == tq:
                maskq[tp * 32 : tp * 32 + NS, tp * 32 : tp * 32 + NS] = (
                    1.0 / (E3SCALE * E3SCALE)
                )
    sc = np.asarray(scale, dtype=f32).reshape(1, 1) / (QSCALE * ALPHA)

    in_maps = []
    for core in range(N_CORES):
        sl = slice(core * T, (core + 1) * T)
        S = np.asarray(support[sl], dtype=f32)        # [16,25,2560]
        Q = np.asarray(query[sl], dtype=f32)          # [16,75,2560]
        lab = np.asarray(support_labels[sl])          # [16,25] int
        # st: S^T x E3SCALE in 32-col strides: [NCH, 128, g*128 + tp*32 + s]
        s64 = np.clip(S * E3SCALE, -15.5, 15.5)
        stp = np.zeros((NCH, 128, G, GP, 32), dtype=f32)
        stp[:, :, :, :, :NS] = (
            s64.reshape(G, GP, NS, NCH, 128).transpose(3, 4, 0, 1, 2)
        )
        st = np.ascontiguousarray(
            stp.reshape(NCH, 128, G * 128).astype(e3)
        )
        sn = np.zeros((G, 128, D), dtype=bf)
        for tp in range(GP):
            sn[:, tp * 32 : tp * 32 + NS, :] = S.reshape(G, GP, NS, D)[:, tp].astype(bf)
        qtp = np.zeros((NCH, 128, NQT), dtype=f32)
        qtp[:, :, : T * NQ] = Q.transpose(2, 0, 1).reshape(NCH, 128, T * NQ)
        if Q_E3:
            qt = np.ascontiguousarray(
                np.clip(qtp * QSCALE, -15.5, 15.5).astype(mybir.dt.np(QDT))
            )
        else:
            qt = np.ascontiguousarray(qtp.astype(bf))
        oh = (lab[:, :, None] == np.arange(NW)[None, None, :]).astype(f32)
        # [16,25,5] -> [128,20]: row = tp*32+s, col = g*5+w
        ohm = np.zeros((128, 20), dtype=f32)
        ohr = oh.reshape(G, GP, NS, NW).transpose(1, 2, 0, 3).reshape(GP, NS, 20)
        for tp in range(GP):
            ohm[tp * 32 : tp * 32 + NS, :] = ohr[tp]
        in_maps.append(
            {
                "st": st,
                "sn": sn,
                "qt": qt,
                "ohc": np.ascontiguousarray(ohm / RHO),
                "h2": np.ascontiguousarray((C_REG + 1.0 / RHO) * ohm),
                "hmo": np.ascontiguousarray(C_REG * ohm),
                "i2": i2,
                "cib": np.ascontiguousarray(ci.astype(bf)),
                "nine": nine,
                "maskq": maskq,
                "scale": sc,
            }
        )
    return in_maps


def _unshard_out(o):
    """[128, OUT_COLS] window layout -> [T, NQ, NW]."""
    logits = np.zeros((T, NQ, NW), dtype=np.float32)
    for (q0, q1, t0, t1, segs, base) in WINDOWS:
        for t in range(t0, t1 + 1):
            r0 = max(0, t * NQ - q0)
            r1 = min(128, (t + 1) * NQ - q0)
            qq0 = q0 + r0 - t * NQ
            logits[t, qq0 : qq0 + (r1 - r0), :] = o[
                r0:r1, base + (t - t0) * NW : base + (t - t0 + 1) * NW
            ]
    return logits


def kernel(query, support, scale, support_labels, n_way, n_shot):
    assert int(n_way) == NW and int(n_shot) * int(n_way) == NS
    assert query.shape == (B_TOT, NQ, D) and support.shape == (B_TOT, NS, D)
    nc = _get_nc()
    in_maps = _host_prep(support, query, support_labels, scale)
    res = run_bass_kernel_spmd(nc, in_maps, core_ids=list(range(N_CORES)))
    outs = []
    for core in range(N_CORES):
        o = np.asarray(res.results[core]["out"])      # [128, OUT_COLS]
        outs.append(_unshard_out(o))
    return np.ascontiguousarray(np.concatenate(outs, axis=0), dtype=np.float32)
